# revision 7
# baseline (speedup 1.0000x reference)
"""LocationAwareAttention Trainium2 kernel.

Full-input contract: kernel(**inputs) takes the complete unsharded inputs
(as produced by the problem's setup_inputs) and returns (context, align) as
full-shape fp32 arrays.  Internally the batch dimension (B=32) is sharded
across 8 NeuronCores (4 batches per core); all weights are replicated.

Math (per batch b):
    conv_feat = conv1d(last_align, conv_w, pad=1) + conv_b          [T, K]
    z[t, a]   = (value[t] @ WV^T)[a] + (conv_feat[t] @ WU^T)[a]
                + (query @ WQ^T)[a] + bias[a]
    score[t]  = fc_w . tanh(z[t])
    p         = sigmoid(score);  align = p / sum(p)
    context   = align @ value                                        [ENC]

Device-side layout (per core, per batch):
  - value is staged host-side as value^T (bf16, [ENC, T]) so the big
    projection runs with the contraction dim (ENC) on partitions.
  - conv+WU collapse into a rank-3 term: u[t,:] = sum_j W3[:,j]*la[t+j-1],
    where W3 = WU @ conv_w — computed on device from WU^T and conv_w.
  - z tiles are [a_chunk(128), t(512)] psum banks: 1 K=3 matmul (u term)
    + 8 K=128 matmuls (value^T @ WV^T chunks); per-batch constant
    q + bias + WU@conv_b enters via the tanh activation's per-partition bias.
  - score: 4 fc matmuls (M=1) accumulate into psum [1, 512]; sigmoid (ACT)
    with accum_out produces the per-tile sum of p for free.
  - context: p broadcast to 128 partitions via a K=1 ones-matmul, then one
    fused DVE tensor_tensor_reduce per e-chunk does mul+row-reduce+chain-
    accumulate into ctx_acc[128, 8].
"""

import os
import sys
import functools

_TRN_REPO = "/opt/trn_rl_repo"
if _TRN_REPO not in sys.path and os.path.isdir(_TRN_REPO):
    sys.path.insert(0, _TRN_REPO)

import numpy as np
import ml_dtypes

BF16 = ml_dtypes.bfloat16

B, T_FULL, DEC, ENC, ATTN, KCONV = 32, 4096, 1024, 1024, 512, 10
N_CORES = 8
BPC = B // N_CORES          # batches per core
TT = 512                    # t-tile (columns per psum bank)
EC = ENC // 128             # e chunks (8)
AC = ATTN // 128            # a chunks (4)


def build_kernel(bpc=BPC, t_len=T_FULL):
    """Build the Bass module for one core handling `bpc` batches of length t_len."""
    import concourse.bass as bass  # noqa: F401
    import concourse.tile as tile
    from concourse import bacc, mybir

    f32 = mybir.dt.float32
    bf16 = mybir.dt.bfloat16
    AF = mybir.ActivationFunctionType
    ALU = mybir.AluOpType

    n_tiles = t_len // TT

    nc = bacc.Bacc(trn_type="TRN2")

    # ---- DRAM I/O ----
    vT = nc.dram_tensor("vT", [bpc, ENC, t_len], bf16, kind="ExternalInput")
    la3 = nc.dram_tensor("la3", [bpc, 3, t_len], bf16, kind="ExternalInput")
    wvt = nc.dram_tensor("wvt", [ENC, ATTN], bf16, kind="ExternalInput")
    wqt = nc.dram_tensor("wqt", [DEC, ATTN], bf16, kind="ExternalInput")
    qT = nc.dram_tensor("qT", [DEC, bpc], bf16, kind="ExternalInput")
    wut = nc.dram_tensor("wut", [KCONV, ATTN], bf16, kind="ExternalInput")
    cw = nc.dram_tensor("cw", [KCONV, 3], bf16, kind="ExternalInput")
    cb = nc.dram_tensor("cb", [KCONV, bpc], bf16, kind="ExternalInput")
    biasT = nc.dram_tensor("biasT", [128, AC], f32, kind="ExternalInput")
    fcr = nc.dram_tensor("fcr", [128, AC], bf16, kind="ExternalInput")

    ctx_out = nc.dram_tensor("ctx_out", [bpc, ENC], f32, kind="ExternalOutput")
    align_out = nc.dram_tensor("align_out", [bpc, t_len], f32, kind="ExternalOutput")

    with tile.TileContext(nc) as tc:
        with (
            tc.tile_pool(name="consts", bufs=1) as consts,
            tc.tile_pool(name="vtp", bufs=3) as vtp,
            tc.tile_pool(name="la3p", bufs=2) as la3p,
            tc.tile_pool(name="tanhp", bufs=8) as tanhp,
            tc.tile_pool(name="pbp", bufs=2) as pbp,
            tc.tile_pool(name="scrp", bufs=2) as scrp,
            tc.tile_pool(name="batchp", bufs=2) as batchp,
            tc.tile_pool(name="psz", bufs=4, space="PSUM") as psz,
            tc.tile_pool(name="pss", bufs=1, space="PSUM") as pss,
            tc.tile_pool(name="psb", bufs=1, space="PSUM") as psb,
            tc.tile_pool(name="psmisc", bufs=2, space="PSUM") as psmisc,
        ):
            # ---- constant loads ----
            wvt_sb = consts.tile([128, EC, ATTN], bf16)
            nc.sync.dma_start(wvt_sb, wvt.rearrange("(c p) a -> p c a", p=128))
            wqt_sb = consts.tile([128, DEC // 128, ATTN], bf16)
            nc.sync.dma_start(wqt_sb, wqt.rearrange("(c p) a -> p c a", p=128))
            qT_sb = consts.tile([128, DEC // 128, bpc], bf16)
            nc.sync.dma_start(qT_sb, qT.rearrange("(c p) b -> p c b", p=128))
            wut_sb = consts.tile([KCONV, ATTN], bf16)
            nc.sync.dma_start(wut_sb, wut[:])
            cw_sb = consts.tile([KCONV, 3], bf16)
            nc.sync.dma_start(cw_sb, cw[:])
            cb_sb = consts.tile([KCONV, bpc], bf16)
            nc.sync.dma_start(cb_sb, cb[:])
            biasT_sb = consts.tile([128, AC], f32)
            nc.sync.dma_start(biasT_sb, biasT[:])
            fcr_sb = consts.tile([128, AC], bf16)
            nc.sync.dma_start(fcr_sb, fcr[:])

            ones_sb = consts.tile([1, 128], f32)
            nc.vector.memset(ones_sb, 1.0)

            # ---- W3T = cw^T @ WU^T : [3, ATTN] ----
            w3_ps = psmisc.tile([3, ATTN], f32, tag="mps")
            nc.tensor.matmul(w3_ps, lhsT=cw_sb, rhs=wut_sb, start=True, stop=True)
            w3_sb = consts.tile([3, ATTN], bf16)
            nc.scalar.copy(w3_sb, w3_ps)

            # ---- cT[a, chunk, b] = (query @ WQ^T + WU @ conv_b + bias)^T ----
            cT_sb = consts.tile([128, AC, bpc], f32)
            for a in range(AC):
                qt_ps = psmisc.tile([128, bpc], f32, tag="mps")
                for c in range(DEC // 128):
                    nc.tensor.matmul(
                        qt_ps,
                        lhsT=wqt_sb[:, c, a * 128:(a + 1) * 128],
                        rhs=qT_sb[:, c, :],
                        start=(c == 0),
                        stop=False,
                    )
                nc.tensor.matmul(
                    qt_ps,
                    lhsT=wut_sb[:, a * 128:(a + 1) * 128],
                    rhs=cb_sb,
                    start=False,
                    stop=True,
                )
                nc.scalar.activation(
                    cT_sb[:, a, :], qt_ps, AF.Identity,
                    bias=biasT_sb[:, a:a + 1], scale=1.0,
                )

            # ---- main loops ----
            for b in range(bpc):
                la3_sb = la3p.tile([3, t_len], bf16)
                nc.sync.dma_start(la3_sb, la3[b])

                p_row = batchp.tile([1, t_len], f32, tag="p_row")
                psums = batchp.tile([1, n_tiles], f32, tag="psums")
                ctx_acc = batchp.tile([128, EC], f32, tag="ctx_acc")
                nc.vector.memset(ctx_acc, 0.0)

                for i in range(n_tiles):
                    tsl = slice(i * TT, (i + 1) * TT)

                    vt_tile = vtp.tile([128, EC, TT], bf16)
                    nc.sync.dma_start(
                        vt_tile,
                        vT[b].rearrange("(c p) t -> p c t", p=128)[:, :, tsl],
                    )

                    tanhs = []
                    for a in range(AC):
                        z_ps = psz.tile([128, TT], f32, tag="z")
                        nc.tensor.matmul(
                            z_ps,
                            lhsT=w3_sb[:, a * 128:(a + 1) * 128],
                            rhs=la3_sb[:, tsl],
                            start=True,
                            stop=False,
                        )
                        for c in range(EC):
                            nc.tensor.matmul(
                                z_ps,
                                lhsT=wvt_sb[:, c, a * 128:(a + 1) * 128],
                                rhs=vt_tile[:, c, :],
                                start=False,
                                stop=(c == EC - 1),
                            )
                        tanh_sb = tanhp.tile([128, TT], bf16, tag="tanh")
                        nc.scalar.activation(
                            tanh_sb, z_ps, AF.Tanh,
                            bias=cT_sb[:, a, b:b + 1], scale=1.0,
                        )
                        tanhs.append(tanh_sb)

                    s_ps = pss.tile([1, TT], f32, tag="s")
                    for a in range(AC):
                        nc.tensor.matmul(
                            s_ps,
                            lhsT=fcr_sb[:, a:a + 1],
                            rhs=tanhs[a],
                            start=(a == 0),
                            stop=(a == AC - 1),
                        )

                    nc.scalar.activation(
                        p_row[0:1, tsl], s_ps, AF.Sigmoid,
                        accum_out=psums[0:1, i:i + 1],
                    )

                    pb_ps = psb.tile([128, TT], f32, tag="pb")
                    nc.tensor.matmul(
                        pb_ps, lhsT=ones_sb, rhs=p_row[0:1, tsl],
                        start=True, stop=True,
                    )
                    pb_sb = pbp.tile([128, TT], bf16, tag="pb_sb")
                    nc.scalar.copy(pb_sb, pb_ps)

                    scr = scrp.tile([128, TT], bf16, tag="scr")
                    for c in range(EC):
                        part = scrp.tile([128, 1], f32, tag="part")
                        nc.vector.scalar_tensor_tensor(
                            out=scr,
                            in0=vt_tile[:, c, :],
                            scalar=1.0,
                            in1=pb_sb,
                            op0=ALU.mult,
                            op1=ALU.mult,
                            accum_out=part,
                        )
                        nc.vector.tensor_add(
                            out=ctx_acc[:, c:c + 1],
                            in0=ctx_acc[:, c:c + 1],
                            in1=part,
                        )

                # ---- batch epilogue ----
                sum1 = batchp.tile([1, 1], f32, tag="sum1")
                nc.vector.reduce_sum(sum1, psums, axis=mybir.AxisListType.X)
                inv1 = batchp.tile([1, 1], f32, tag="inv1")
                nc.vector.reciprocal(inv1, sum1)

                align_sb = batchp.tile([1, t_len], f32, tag="align_sb")
                nc.scalar.activation(
                    align_sb, p_row, AF.Copy, scale=inv1[0:1, 0:1]
                )
                nc.sync.dma_start(align_out[b:b + 1, :], align_sb)

                invp_ps = psmisc.tile([128, 1], f32, tag="mps")
                nc.tensor.matmul(invp_ps, lhsT=ones_sb, rhs=inv1, start=True, stop=True)
                invp_sb = batchp.tile([128, 1], f32, tag="invp_sb")
                nc.scalar.copy(invp_sb, invp_ps)

                ctx_sb = batchp.tile([128, EC], f32, tag="ctx_sb")
                nc.vector.tensor_scalar_mul(ctx_sb, ctx_acc, invp_sb)
                nc.sync.dma_start(
                    ctx_out[b].rearrange("(c p) -> p c", p=128), ctx_sb
                )

    nc.compile()
    return nc


def prep_inputs(query, value, last_align, conv_w, conv_b, WQ, WV, WU, bias, fc_w,
                bpc=BPC, n_cores=N_CORES):
    """Host-side sharding + layout prep. Returns list of per-core input dicts."""
    t_len = value.shape[1]

    value = np.asarray(value, np.float32)
    la = np.asarray(last_align, np.float32)

    # value^T per batch, bf16
    vT_all = np.ascontiguousarray(value.transpose(0, 2, 1)).astype(BF16)

    nb = vT_all.shape[0]
    la3_all = np.zeros((nb, 3, t_len), np.float32)
    la3_all[:, 0, 1:] = la[:, :-1]
    la3_all[:, 1, :] = la
    la3_all[:, 2, :-1] = la[:, 1:]
    la3_all = la3_all.astype(BF16)

    wvt_h = np.ascontiguousarray(np.asarray(WV, np.float32).T).astype(BF16)
    wqt_h = np.ascontiguousarray(np.asarray(WQ, np.float32).T).astype(BF16)
    wut_h = np.ascontiguousarray(np.asarray(WU, np.float32).T).astype(BF16)
    cw_h = np.ascontiguousarray(np.asarray(conv_w, np.float32)[:, 0, :]).astype(BF16)
    biasT_h = np.ascontiguousarray(
        np.asarray(bias, np.float32).reshape(AC, 128).T
    ).astype(np.float32)
    fcr_h = np.ascontiguousarray(
        np.asarray(fc_w, np.float32)[0].reshape(AC, 128).T
    ).astype(BF16)

    qT_all = np.ascontiguousarray(
        np.asarray(query, np.float32)[:, 0, :].T
    ).astype(BF16)  # [DEC, B]
    cb_h = np.ascontiguousarray(
        np.repeat(np.asarray(conv_b, np.float32)[:, None], bpc, axis=1)
    ).astype(BF16)

    in_maps = []
    for c in range(n_cores):
        bs = slice(c * bpc, (c + 1) * bpc)
        in_maps.append({
            "vT": np.ascontiguousarray(vT_all[bs]),
            "la3": np.ascontiguousarray(la3_all[bs]),
            "wvt": wvt_h,
            "wqt": wqt_h,
            "qT": np.ascontiguousarray(qT_all[:, bs]),
            "wut": wut_h,
            "cw": cw_h,
            "cb": cb_h,
            "biasT": biasT_h,
            "fcr": fcr_h,
        })
    return in_maps


@functools.lru_cache(maxsize=1)
def _get_nc():
    return build_kernel()


def run(inputs, trace=False, **kwargs):
    from concourse.bass_utils import run_bass_kernel_spmd

    nc = _get_nc()
    in_maps = prep_inputs(**inputs)
    res = run_bass_kernel_spmd(
        nc, in_maps, core_ids=list(range(N_CORES)), trace=trace, **kwargs
    )
    ctx = np.concatenate([np.asarray(r["ctx_out"]) for r in res.results], axis=0)
    align = np.concatenate([np.asarray(r["align_out"]) for r in res.results], axis=0)
    return (ctx.astype(np.float32), align.astype(np.float32)), res


def kernel(**inputs):
    (ctx, align), _ = run(inputs, trace=False)
    return ctx, align


# revision 13
# speedup vs baseline: 1.0930x; 1.0930x over previous
"""LocationAwareAttention Trainium2 kernel.

Full-input contract: kernel(**inputs) takes the complete unsharded inputs
(as produced by the problem's setup_inputs) and returns (context, align) as
full-shape fp32 arrays.  Internally the batch dimension (B=32) is sharded
across 8 NeuronCores (4 batches per core); all weights are replicated.

Math (per batch b):
    conv_feat = conv1d(last_align, conv_w, pad=1) + conv_b          [T, K]
    z[t, a]   = (value[t] @ WV^T)[a] + (conv_feat[t] @ WU^T)[a]
                + (query @ WQ^T)[a] + bias[a]
    score[t]  = fc_w . tanh(z[t])
    p         = sigmoid(score);  align = p / sum(p)
    context   = align @ value                                        [ENC]

Device-side layout (per core, per batch):
  - value is staged host-side as value^T (bf16, [ENC, T]) so the big
    projection runs with the contraction dim (ENC) on partitions.
  - conv+WU collapse into a rank-3 term: u[t,:] = sum_j W3[:,j]*la[t+j-1],
    where W3 = WU @ conv_w — computed on device from WU^T and conv_w.
  - z tiles are [a_chunk(128), t(512)] psum banks: 1 K=3 matmul (u term)
    + 8 K=128 matmuls (value^T @ WV^T chunks); per-batch constant
    q + bias + WU@conv_b enters via the tanh activation's per-partition bias.
  - score: 4 fc matmuls (M=1) accumulate into psum [1, 512]; sigmoid (ACT)
    with accum_out produces the per-tile sum of p for free.
  - context: p broadcast to 128 partitions via a K=1 ones-matmul, then one
    fused DVE tensor_tensor_reduce per e-chunk does mul+row-reduce+chain-
    accumulate into ctx_acc[128, 8].
"""

import os
import sys
import functools

_TRN_REPO = "/opt/trn_rl_repo"
if _TRN_REPO not in sys.path and os.path.isdir(_TRN_REPO):
    sys.path.insert(0, _TRN_REPO)

import numpy as np
import ml_dtypes

BF16 = ml_dtypes.bfloat16

B, T_FULL, DEC, ENC, ATTN, KCONV = 32, 4096, 1024, 1024, 512, 10
N_CORES = 8
BPC = B // N_CORES          # batches per core
TT = 512                    # t-tile (columns per psum bank)
EC = ENC // 128             # e chunks (8)
AC = ATTN // 128            # a chunks (4)


def build_kernel(bpc=BPC, t_len=T_FULL):
    """Build the Bass module for one core handling `bpc` batches of length t_len."""
    import concourse.bass as bass  # noqa: F401
    import concourse.tile as tile
    from concourse import bacc, mybir

    f32 = mybir.dt.float32
    bf16 = mybir.dt.bfloat16
    AF = mybir.ActivationFunctionType
    ALU = mybir.AluOpType

    n_tiles = t_len // TT

    nc = bacc.Bacc(trn_type="TRN2")

    # ---- DRAM I/O ----
    vT = nc.dram_tensor("vT", [bpc, ENC, t_len], bf16, kind="ExternalInput")
    la3 = nc.dram_tensor("la3", [bpc, 3, t_len], bf16, kind="ExternalInput")
    wvt = nc.dram_tensor("wvt", [ENC, ATTN], bf16, kind="ExternalInput")
    wqt = nc.dram_tensor("wqt", [DEC, ATTN], bf16, kind="ExternalInput")
    qT = nc.dram_tensor("qT", [DEC, bpc], bf16, kind="ExternalInput")
    wut = nc.dram_tensor("wut", [KCONV, ATTN], bf16, kind="ExternalInput")
    cw = nc.dram_tensor("cw", [KCONV, 3], bf16, kind="ExternalInput")
    cb = nc.dram_tensor("cb", [KCONV, bpc], bf16, kind="ExternalInput")
    biasT = nc.dram_tensor("biasT", [128, AC], f32, kind="ExternalInput")
    fcr = nc.dram_tensor("fcr", [128, AC], bf16, kind="ExternalInput")

    ctx_out = nc.dram_tensor("ctx_out", [bpc, ENC], f32, kind="ExternalOutput")
    align_out = nc.dram_tensor("align_out", [bpc, t_len], f32, kind="ExternalOutput")

    with tile.TileContext(nc) as tc:
        with (
            tc.tile_pool(name="consts", bufs=1) as consts,
            tc.tile_pool(name="vtp", bufs=2) as vtp,
            tc.tile_pool(name="la3p", bufs=2) as la3p,
            tc.tile_pool(name="tanhp", bufs=20) as tanhp,
            tc.tile_pool(name="pbp", bufs=3) as pbp,
            tc.tile_pool(name="scrp", bufs=2) as scrp,
            tc.tile_pool(name="batchp", bufs=2) as batchp,
            tc.tile_pool(name="dramp", bufs=2, space="DRAM") as dramp,
            tc.tile_pool(name="psz", bufs=5, space="PSUM") as psz,
            tc.tile_pool(name="pss", bufs=2, space="PSUM") as pss,
            tc.tile_pool(name="psmisc", bufs=1, space="PSUM") as psmisc,
        ):
            # ---- constant loads ----
            wvt_sb = consts.tile([128, EC, ATTN], bf16)
            nc.sync.dma_start(wvt_sb, wvt.rearrange("(c p) a -> p c a", p=128))
            wqt_sb = consts.tile([128, DEC // 128, ATTN], bf16)
            nc.sync.dma_start(wqt_sb, wqt.rearrange("(c p) a -> p c a", p=128))
            qT_sb = consts.tile([128, DEC // 128, bpc], bf16)
            nc.sync.dma_start(qT_sb, qT.rearrange("(c p) b -> p c b", p=128))
            wut_sb = consts.tile([KCONV, ATTN], bf16)
            nc.sync.dma_start(wut_sb, wut[:])
            cw_sb = consts.tile([KCONV, 3], bf16)
            nc.sync.dma_start(cw_sb, cw[:])
            cb_sb = consts.tile([KCONV, bpc], bf16)
            nc.sync.dma_start(cb_sb, cb[:])
            biasT_sb = consts.tile([128, AC], f32)
            nc.sync.dma_start(biasT_sb, biasT[:])
            fcr_sb = consts.tile([128, AC], bf16)
            nc.sync.dma_start(fcr_sb, fcr[:])

            ones_sb = consts.tile([1, 128], f32)
            nc.vector.memset(ones_sb, 1.0)

            # ---- W3T = cw^T @ WU^T : [3, ATTN] ----
            w3_ps = psmisc.tile([3, ATTN], f32, tag="mps")
            nc.tensor.matmul(w3_ps, lhsT=cw_sb, rhs=wut_sb, start=True, stop=True)
            w3_sb = consts.tile([3, ATTN], bf16)
            nc.scalar.copy(w3_sb, w3_ps)

            # ---- cT[a, chunk, b] = (query @ WQ^T + WU @ conv_b + bias)^T ----
            cT_sb = consts.tile([128, AC, bpc], f32)
            for a in range(AC):
                qt_ps = psmisc.tile([128, bpc], f32, tag="mps")
                for c in range(DEC // 128):
                    nc.tensor.matmul(
                        qt_ps,
                        lhsT=wqt_sb[:, c, a * 128:(a + 1) * 128],
                        rhs=qT_sb[:, c, :],
                        start=(c == 0),
                        stop=False,
                    )
                nc.tensor.matmul(
                    qt_ps,
                    lhsT=wut_sb[:, a * 128:(a + 1) * 128],
                    rhs=cb_sb,
                    start=False,
                    stop=True,
                )
                nc.scalar.activation(
                    cT_sb[:, a, :], qt_ps, AF.Identity,
                    bias=biasT_sb[:, a:a + 1], scale=1.0,
                )

            # ---- main loops ----
            JJ = min(4, n_tiles)        # t512 subtiles per matmul group
            GTT = JJ * TT               # 2048 columns per vT DMA tile
            n_groups = t_len // GTT
            for b in range(bpc):
                la3_sb = la3p.tile([3, t_len], bf16)
                nc.sync.dma_start(la3_sb, la3[b])

                p_row = batchp.tile([1, t_len], f32, tag="p_row")
                p_dram = dramp.tile([t_len], f32, tag="p_dram")
                psums = batchp.tile([1, n_tiles], f32, tag="psums")
                parts = batchp.tile([128, EC, n_tiles], f32, tag="parts")

                for ip in range(n_groups):
                    gsl = slice(ip * GTT, (ip + 1) * GTT)
                    vt_tile = vtp.tile([128, EC, GTT], bf16)
                    nc.sync.dma_start(
                        vt_tile,
                        vT[b].rearrange("(c p) t -> p c t", p=128)[:, :, gsl],
                    )

                    tanhs = {}
                    for a in range(AC):
                        zs = []
                        for j in range(JJ):
                            z_ps = psz.tile([128, TT], f32, tag="z",
                                            name=f"z_{b}_{ip}_{a}_{j}")
                            zs.append(z_ps)
                        for j in range(JJ):
                            nc.tensor.matmul(
                                zs[j],
                                lhsT=w3_sb[:, a * 128:(a + 1) * 128],
                                rhs=la3_sb[:, (ip * JJ + j) * TT:
                                           (ip * JJ + j + 1) * TT],
                                start=True,
                                stop=False,
                            )
                        for c in range(EC):
                            for j in range(JJ):
                                nc.tensor.matmul(
                                    zs[j],
                                    lhsT=wvt_sb[:, c, a * 128:(a + 1) * 128],
                                    rhs=vt_tile[:, c, j * TT:(j + 1) * TT],
                                    start=False,
                                    stop=(c == EC - 1),
                                )
                        for j in range(JJ):
                            tanh_sb = tanhp.tile([128, TT], bf16, tag="tanh",
                                                 name=f"tanh_{b}_{ip}_{a}_{j}")
                            nc.scalar.activation(
                                tanh_sb, zs[j], AF.Tanh,
                                bias=cT_sb[:, a, b:b + 1], scale=1.0,
                            )
                            tanhs[(a, j)] = tanh_sb

                    for j in range(JJ):
                        ti = ip * JJ + j
                        tsl = slice(ti * TT, (ti + 1) * TT)
                        s_ps = pss.tile([1, TT], f32, tag="s",
                                        name=f"s_{b}_{ip}_{j}")
                        for a in range(AC):
                            nc.tensor.matmul(
                                s_ps,
                                lhsT=fcr_sb[:, a:a + 1],
                                rhs=tanhs[(a, j)],
                                start=(a == 0),
                                stop=(a == AC - 1),
                            )

                        nc.scalar.activation(
                            p_row[0:1, tsl], s_ps, AF.Sigmoid,
                            accum_out=psums[0:1, ti:ti + 1],
                        )

                        nc.sync.dma_start(
                            p_dram[tsl][None, :], p_row[0:1, tsl]
                        )
                        pb_sb = pbp.tile([128, TT], bf16, tag="pb_sb",
                                         name=f"pb_{b}_{ip}_{j}")
                        nc.gpsimd.dma_start(
                            pb_sb,
                            p_dram[tsl][None, :].to_broadcast([128, TT]),
                        )

                        scr = scrp.tile([128, TT], bf16, tag="scr",
                                        name=f"scr_{b}_{ip}_{j}")
                        for c in range(EC):
                            nc.vector.scalar_tensor_tensor(
                                out=scr,
                                in0=vt_tile[:, c, j * TT:(j + 1) * TT],
                                scalar=1.0,
                                in1=pb_sb,
                                op0=ALU.mult,
                                op1=ALU.mult,
                                accum_out=parts[:, c, ti:ti + 1],
                            )

                # ---- batch epilogue ----
                sum1 = batchp.tile([1, 1], f32, tag="sum1")
                nc.vector.reduce_sum(sum1, psums, axis=mybir.AxisListType.X)
                inv1 = batchp.tile([1, 1], f32, tag="inv1")
                nc.vector.reciprocal(inv1, sum1)

                align_sb = batchp.tile([1, t_len], f32, tag="align_sb")
                nc.scalar.activation(
                    align_sb, p_row, AF.Copy, scale=inv1[0:1, 0:1]
                )
                nc.sync.dma_start(align_out[b:b + 1, :], align_sb)

                invp_ps = psmisc.tile([128, 1], f32, tag="mps",
                                      name=f"invp_{b}")
                nc.tensor.matmul(invp_ps, lhsT=ones_sb, rhs=inv1, start=True, stop=True)
                invp_sb = batchp.tile([128, 1], f32, tag="invp_sb")
                nc.scalar.copy(invp_sb, invp_ps)

                ctx_acc = batchp.tile([128, EC], f32, tag="ctx_acc")
                for c in range(EC):
                    nc.vector.reduce_sum(
                        ctx_acc[:, c:c + 1], parts[:, c, :],
                        axis=mybir.AxisListType.X,
                    )
                ctx_sb = batchp.tile([128, EC], f32, tag="ctx_sb")
                nc.vector.tensor_scalar_mul(ctx_sb, ctx_acc, invp_sb)
                nc.sync.dma_start(
                    ctx_out[b].rearrange("(c p) -> p c", p=128), ctx_sb
                )

    nc.compile()
    return nc


def prep_inputs(query, value, last_align, conv_w, conv_b, WQ, WV, WU, bias, fc_w,
                bpc=BPC, n_cores=N_CORES):
    """Host-side sharding + layout prep. Returns list of per-core input dicts."""
    t_len = value.shape[1]

    value = np.asarray(value, np.float32)
    la = np.asarray(last_align, np.float32)

    # value^T per batch, bf16
    vT_all = np.ascontiguousarray(value.transpose(0, 2, 1)).astype(BF16)

    nb = vT_all.shape[0]
    la3_all = np.zeros((nb, 3, t_len), np.float32)
    la3_all[:, 0, 1:] = la[:, :-1]
    la3_all[:, 1, :] = la
    la3_all[:, 2, :-1] = la[:, 1:]
    la3_all = la3_all.astype(BF16)

    wvt_h = np.ascontiguousarray(np.asarray(WV, np.float32).T).astype(BF16)
    wqt_h = np.ascontiguousarray(np.asarray(WQ, np.float32).T).astype(BF16)
    wut_h = np.ascontiguousarray(np.asarray(WU, np.float32).T).astype(BF16)
    cw_h = np.ascontiguousarray(np.asarray(conv_w, np.float32)[:, 0, :]).astype(BF16)
    biasT_h = np.ascontiguousarray(
        np.asarray(bias, np.float32).reshape(AC, 128).T
    ).astype(np.float32)
    fcr_h = np.ascontiguousarray(
        np.asarray(fc_w, np.float32)[0].reshape(AC, 128).T
    ).astype(BF16)

    qT_all = np.ascontiguousarray(
        np.asarray(query, np.float32)[:, 0, :].T
    ).astype(BF16)  # [DEC, B]
    cb_h = np.ascontiguousarray(
        np.repeat(np.asarray(conv_b, np.float32)[:, None], bpc, axis=1)
    ).astype(BF16)

    in_maps = []
    for c in range(n_cores):
        bs = slice(c * bpc, (c + 1) * bpc)
        in_maps.append({
            "vT": np.ascontiguousarray(vT_all[bs]),
            "la3": np.ascontiguousarray(la3_all[bs]),
            "wvt": wvt_h,
            "wqt": wqt_h,
            "qT": np.ascontiguousarray(qT_all[:, bs]),
            "wut": wut_h,
            "cw": cw_h,
            "cb": cb_h,
            "biasT": biasT_h,
            "fcr": fcr_h,
        })
    return in_maps


@functools.lru_cache(maxsize=1)
def _get_nc():
    return build_kernel()


def run(inputs, trace=False, **kwargs):
    from concourse.bass_utils import run_bass_kernel_spmd

    nc = _get_nc()
    in_maps = prep_inputs(**inputs)
    res = run_bass_kernel_spmd(
        nc, in_maps, core_ids=list(range(N_CORES)), trace=trace, **kwargs
    )
    ctx = np.concatenate([np.asarray(r["ctx_out"]) for r in res.results], axis=0)
    align = np.concatenate([np.asarray(r["align_out"]) for r in res.results], axis=0)
    return (ctx.astype(np.float32), align.astype(np.float32)), res


def kernel(**inputs):
    (ctx, align), _ = run(inputs, trace=False)
    return ctx, align


# revision 15
# speedup vs baseline: 1.1716x; 1.0719x over previous
"""LocationAwareAttention Trainium2 kernel.

Full-input contract: kernel(**inputs) takes the complete unsharded inputs
(as produced by the problem's setup_inputs) and returns (context, align) as
full-shape fp32 arrays.  Internally the batch dimension (B=32) is sharded
across 8 NeuronCores (4 batches per core); all weights are replicated.

Math (per batch b):
    conv_feat = conv1d(last_align, conv_w, pad=1) + conv_b          [T, K]
    z[t, a]   = (value[t] @ WV^T)[a] + (conv_feat[t] @ WU^T)[a]
                + (query @ WQ^T)[a] + bias[a]
    score[t]  = fc_w . tanh(z[t])
    p         = sigmoid(score);  align = p / sum(p)
    context   = align @ value                                        [ENC]

Device-side layout (per core, per batch):
  - value is staged host-side as value^T (bf16, [ENC, T]) so the big
    projection runs with the contraction dim (ENC) on partitions.
  - conv+WU collapse into a rank-3 term: u[t,:] = sum_j W3[:,j]*la[t+j-1],
    where W3 = WU @ conv_w — computed on device from WU^T and conv_w.
  - z tiles are [a_chunk(128), t(512)] psum banks: 1 K=3 matmul (u term)
    + 8 K=128 matmuls (value^T @ WV^T chunks); per-batch constant
    q + bias + WU@conv_b enters via the tanh activation's per-partition bias.
  - score: 4 fc matmuls (M=1) accumulate into psum [1, 512]; sigmoid (ACT)
    with accum_out produces the per-tile sum of p for free.
  - context: p broadcast to 128 partitions via a K=1 ones-matmul, then one
    fused DVE tensor_tensor_reduce per e-chunk does mul+row-reduce+chain-
    accumulate into ctx_acc[128, 8].
"""

import os
import sys
import functools

_TRN_REPO = "/opt/trn_rl_repo"
if _TRN_REPO not in sys.path and os.path.isdir(_TRN_REPO):
    sys.path.insert(0, _TRN_REPO)

import numpy as np
import ml_dtypes

BF16 = ml_dtypes.bfloat16

B, T_FULL, DEC, ENC, ATTN, KCONV = 32, 4096, 1024, 1024, 512, 10
N_CORES = 8
BPC = B // N_CORES          # batches per core
TT = 512                    # t-tile (columns per psum bank)
EC = ENC // 128             # e chunks (8)
AC = ATTN // 128            # a chunks (4)


def build_kernel(bpc=BPC, t_len=T_FULL):
    """Build the Bass module for one core handling `bpc` batches of length t_len."""
    import concourse.bass as bass  # noqa: F401
    import concourse.tile as tile
    from concourse import bacc, mybir

    f32 = mybir.dt.float32
    bf16 = mybir.dt.bfloat16
    AF = mybir.ActivationFunctionType
    ALU = mybir.AluOpType

    n_tiles = t_len // TT

    nc = bacc.Bacc(trn_type="TRN2")

    # ---- DRAM I/O ----
    vT = nc.dram_tensor("vT", [bpc, ENC, t_len], bf16, kind="ExternalInput")
    la3 = nc.dram_tensor("la3", [bpc, 3, t_len], bf16, kind="ExternalInput")
    wvt = nc.dram_tensor("wvt", [ENC, ATTN], bf16, kind="ExternalInput")
    wqt = nc.dram_tensor("wqt", [DEC, ATTN], bf16, kind="ExternalInput")
    qT = nc.dram_tensor("qT", [DEC, bpc], bf16, kind="ExternalInput")
    wut = nc.dram_tensor("wut", [KCONV, ATTN], bf16, kind="ExternalInput")
    cw = nc.dram_tensor("cw", [KCONV, 3], bf16, kind="ExternalInput")
    cb = nc.dram_tensor("cb", [KCONV, bpc], bf16, kind="ExternalInput")
    biasT = nc.dram_tensor("biasT", [128, AC], f32, kind="ExternalInput")
    fcr = nc.dram_tensor("fcr", [128, AC], bf16, kind="ExternalInput")

    ctx_out = nc.dram_tensor("ctx_out", [bpc, ENC], f32, kind="ExternalOutput")
    align_out = nc.dram_tensor("align_out", [bpc, t_len], f32, kind="ExternalOutput")

    with tile.TileContext(nc) as tc:
        with (
            tc.tile_pool(name="consts", bufs=1) as consts,
            tc.tile_pool(name="setupp", bufs=1) as setupp,
            tc.tile_pool(name="vtp", bufs=3) as vtp,
            tc.tile_pool(name="la3p", bufs=2) as la3p,
            tc.tile_pool(name="tanhp", bufs=20) as tanhp,
            tc.tile_pool(name="pbp", bufs=3) as pbp,
            tc.tile_pool(name="scrp", bufs=2) as scrp,
            tc.tile_pool(name="batchp", bufs=2) as batchp,
            tc.tile_pool(name="dramp", bufs=2, space="DRAM") as dramp,
            tc.tile_pool(name="psz", bufs=5, space="PSUM") as psz,
            tc.tile_pool(name="pss", bufs=2, space="PSUM") as pss,
            tc.tile_pool(name="psmisc", bufs=1, space="PSUM") as psmisc,
        ):
            # ---- constant loads ----
            wvt_sb = consts.tile([128, EC, ATTN], bf16)
            nc.sync.dma_start(wvt_sb, wvt.rearrange("(c p) a -> p c a", p=128))
            wqt_sb = setupp.tile([128, DEC // 128, ATTN], bf16)
            nc.sync.dma_start(wqt_sb, wqt.rearrange("(c p) a -> p c a", p=128))
            qT_sb = setupp.tile([128, DEC // 128, bpc], bf16)
            nc.sync.dma_start(qT_sb, qT.rearrange("(c p) b -> p c b", p=128))
            wut_sb = consts.tile([KCONV, ATTN], bf16)
            nc.sync.dma_start(wut_sb, wut[:])
            cw_sb = consts.tile([KCONV, 3], bf16)
            nc.sync.dma_start(cw_sb, cw[:])
            cb_sb = consts.tile([KCONV, bpc], bf16)
            nc.sync.dma_start(cb_sb, cb[:])
            biasT_sb = consts.tile([128, AC], f32)
            nc.sync.dma_start(biasT_sb, biasT[:])
            fcr_sb = consts.tile([128, AC], bf16)
            nc.sync.dma_start(fcr_sb, fcr[:])

            ones_sb = consts.tile([1, 128], f32)
            nc.vector.memset(ones_sb, 1.0)

            # ---- W3T = cw^T @ WU^T : [3, ATTN] ----
            w3_ps = psmisc.tile([3, ATTN], f32, tag="mps")
            nc.tensor.matmul(w3_ps, lhsT=cw_sb, rhs=wut_sb, start=True, stop=True)
            w3_sb = consts.tile([3, ATTN], bf16)
            nc.scalar.copy(w3_sb, w3_ps)

            # ---- cT[a, chunk, b] = (query @ WQ^T + WU @ conv_b + bias)^T ----
            cT_sb = consts.tile([128, AC, bpc], f32)
            for a in range(AC):
                qt_ps = psmisc.tile([128, bpc], f32, tag="mps")
                for c in range(DEC // 128):
                    nc.tensor.matmul(
                        qt_ps,
                        lhsT=wqt_sb[:, c, a * 128:(a + 1) * 128],
                        rhs=qT_sb[:, c, :],
                        start=(c == 0),
                        stop=False,
                    )
                nc.tensor.matmul(
                    qt_ps,
                    lhsT=wut_sb[:, a * 128:(a + 1) * 128],
                    rhs=cb_sb,
                    start=False,
                    stop=True,
                )
                nc.scalar.activation(
                    cT_sb[:, a, :], qt_ps, AF.Identity,
                    bias=biasT_sb[:, a:a + 1], scale=1.0,
                )

            # ---- main loops ----
            JJ = min(4, n_tiles)        # t512 subtiles per matmul group
            GTT = JJ * TT               # 2048 columns per vT DMA tile
            n_groups = t_len // GTT
            for b in range(bpc):
                la3_sb = la3p.tile([3, t_len], bf16)
                nc.sync.dma_start(la3_sb, la3[b])

                p_row = batchp.tile([1, t_len], f32, tag="p_row")
                p_dram = dramp.tile([t_len], f32, tag="p_dram")
                psums = batchp.tile([1, n_tiles], f32, tag="psums")
                parts = batchp.tile([128, EC, n_tiles], f32, tag="parts")

                for ip in range(n_groups):
                    gsl = slice(ip * GTT, (ip + 1) * GTT)
                    vt_tile = vtp.tile([128, EC, GTT], bf16)
                    nc.sync.dma_start(
                        vt_tile,
                        vT[b].rearrange("(c p) t -> p c t", p=128)[:, :, gsl],
                    )

                    tanhs = {}
                    for a in range(AC):
                        zs = []
                        for j in range(JJ):
                            z_ps = psz.tile([128, TT], f32, tag="z",
                                            name=f"z_{b}_{ip}_{a}_{j}")
                            zs.append(z_ps)
                        for j in range(JJ):
                            nc.tensor.matmul(
                                zs[j],
                                lhsT=w3_sb[:, a * 128:(a + 1) * 128],
                                rhs=la3_sb[:, (ip * JJ + j) * TT:
                                           (ip * JJ + j + 1) * TT],
                                start=True,
                                stop=False,
                            )
                        for c in range(EC):
                            for j in range(JJ):
                                nc.tensor.matmul(
                                    zs[j],
                                    lhsT=wvt_sb[:, c, a * 128:(a + 1) * 128],
                                    rhs=vt_tile[:, c, j * TT:(j + 1) * TT],
                                    start=False,
                                    stop=(c == EC - 1),
                                )
                        for j in range(JJ):
                            tanh_sb = tanhp.tile([128, TT], bf16, tag="tanh",
                                                 name=f"tanh_{b}_{ip}_{a}_{j}")
                            nc.scalar.activation(
                                tanh_sb, zs[j], AF.Tanh,
                                bias=cT_sb[:, a, b:b + 1], scale=1.0,
                            )
                            tanhs[(a, j)] = tanh_sb

                    for j in range(JJ):
                        ti = ip * JJ + j
                        tsl = slice(ti * TT, (ti + 1) * TT)
                        s_ps = pss.tile([1, TT], f32, tag="s",
                                        name=f"s_{b}_{ip}_{j}")
                        for a in range(AC):
                            nc.tensor.matmul(
                                s_ps,
                                lhsT=fcr_sb[:, a:a + 1],
                                rhs=tanhs[(a, j)],
                                start=(a == 0),
                                stop=(a == AC - 1),
                            )

                        nc.scalar.activation(
                            p_row[0:1, tsl], s_ps, AF.Sigmoid,
                            accum_out=psums[0:1, ti:ti + 1],
                        )

                        nc.scalar.dma_start(
                            p_dram[tsl][None, :], p_row[0:1, tsl]
                        )
                        pb_sb = pbp.tile([128, TT], bf16, tag="pb_sb",
                                         name=f"pb_{b}_{ip}_{j}")
                        nc.gpsimd.dma_start(
                            pb_sb,
                            p_dram[tsl][None, :].to_broadcast([128, TT]),
                        )

                        scr = scrp.tile([128, TT], bf16, tag="scr",
                                        name=f"scr_{b}_{ip}_{j}")
                        for c in range(EC):
                            nc.vector.scalar_tensor_tensor(
                                out=scr,
                                in0=vt_tile[:, c, j * TT:(j + 1) * TT],
                                scalar=1.0,
                                in1=pb_sb,
                                op0=ALU.mult,
                                op1=ALU.mult,
                                accum_out=parts[:, c, ti:ti + 1],
                            )

                # ---- batch epilogue ----
                sum1 = batchp.tile([1, 1], f32, tag="sum1")
                nc.vector.reduce_sum(sum1, psums, axis=mybir.AxisListType.X)
                inv1 = batchp.tile([1, 1], f32, tag="inv1")
                nc.vector.reciprocal(inv1, sum1)

                nc.scalar.activation(
                    p_row, p_row, AF.Copy, scale=inv1[0:1, 0:1]
                )
                nc.scalar.dma_start(align_out[b:b + 1, :], p_row)

                invp_ps = psmisc.tile([128, 1], f32, tag="mps",
                                      name=f"invp_{b}")
                nc.tensor.matmul(invp_ps, lhsT=ones_sb, rhs=inv1, start=True, stop=True)
                invp_sb = batchp.tile([128, 1], f32, tag="invp_sb")
                nc.scalar.copy(invp_sb, invp_ps)

                ctx_acc = batchp.tile([128, EC], f32, tag="ctx_acc")
                for c in range(EC):
                    nc.vector.reduce_sum(
                        ctx_acc[:, c:c + 1], parts[:, c, :],
                        axis=mybir.AxisListType.X,
                    )
                ctx_sb = batchp.tile([128, EC], f32, tag="ctx_sb")
                nc.vector.tensor_scalar_mul(ctx_sb, ctx_acc, invp_sb)
                nc.scalar.dma_start(
                    ctx_out[b].rearrange("(c p) -> p c", p=128), ctx_sb
                )

    nc.compile()
    return nc


def prep_inputs(query, value, last_align, conv_w, conv_b, WQ, WV, WU, bias, fc_w,
                bpc=BPC, n_cores=N_CORES):
    """Host-side sharding + layout prep. Returns list of per-core input dicts."""
    t_len = value.shape[1]

    value = np.asarray(value, np.float32)
    la = np.asarray(last_align, np.float32)

    # value^T per batch, bf16
    vT_all = np.ascontiguousarray(value.transpose(0, 2, 1)).astype(BF16)

    nb = vT_all.shape[0]
    la3_all = np.zeros((nb, 3, t_len), np.float32)
    la3_all[:, 0, 1:] = la[:, :-1]
    la3_all[:, 1, :] = la
    la3_all[:, 2, :-1] = la[:, 1:]
    la3_all = la3_all.astype(BF16)

    wvt_h = np.ascontiguousarray(np.asarray(WV, np.float32).T).astype(BF16)
    wqt_h = np.ascontiguousarray(np.asarray(WQ, np.float32).T).astype(BF16)
    wut_h = np.ascontiguousarray(np.asarray(WU, np.float32).T).astype(BF16)
    cw_h = np.ascontiguousarray(np.asarray(conv_w, np.float32)[:, 0, :]).astype(BF16)
    biasT_h = np.ascontiguousarray(
        np.asarray(bias, np.float32).reshape(AC, 128).T
    ).astype(np.float32)
    fcr_h = np.ascontiguousarray(
        np.asarray(fc_w, np.float32)[0].reshape(AC, 128).T
    ).astype(BF16)

    qT_all = np.ascontiguousarray(
        np.asarray(query, np.float32)[:, 0, :].T
    ).astype(BF16)  # [DEC, B]
    cb_h = np.ascontiguousarray(
        np.repeat(np.asarray(conv_b, np.float32)[:, None], bpc, axis=1)
    ).astype(BF16)

    in_maps = []
    for c in range(n_cores):
        bs = slice(c * bpc, (c + 1) * bpc)
        in_maps.append({
            "vT": np.ascontiguousarray(vT_all[bs]),
            "la3": np.ascontiguousarray(la3_all[bs]),
            "wvt": wvt_h,
            "wqt": wqt_h,
            "qT": np.ascontiguousarray(qT_all[:, bs]),
            "wut": wut_h,
            "cw": cw_h,
            "cb": cb_h,
            "biasT": biasT_h,
            "fcr": fcr_h,
        })
    return in_maps


@functools.lru_cache(maxsize=1)
def _get_nc():
    return build_kernel()


def run(inputs, trace=False, **kwargs):
    from concourse.bass_utils import run_bass_kernel_spmd

    nc = _get_nc()
    in_maps = prep_inputs(**inputs)
    res = run_bass_kernel_spmd(
        nc, in_maps, core_ids=list(range(N_CORES)), trace=trace, **kwargs
    )
    ctx = np.concatenate([np.asarray(r["ctx_out"]) for r in res.results], axis=0)
    align = np.concatenate([np.asarray(r["align_out"]) for r in res.results], axis=0)
    return (ctx.astype(np.float32), align.astype(np.float32)), res


def kernel(**inputs):
    (ctx, align), _ = run(inputs, trace=False)
    return ctx, align


# revision 16
# speedup vs baseline: 1.2607x; 1.0761x over previous
"""LocationAwareAttention Trainium2 kernel.

Full-input contract: kernel(**inputs) takes the complete unsharded inputs
(as produced by the problem's setup_inputs) and returns (context, align) as
full-shape fp32 arrays.  Internally the batch dimension (B=32) is sharded
across 8 NeuronCores (4 batches per core); all weights are replicated.

Math (per batch b):
    conv_feat = conv1d(last_align, conv_w, pad=1) + conv_b          [T, K]
    z[t, a]   = (value[t] @ WV^T)[a] + (conv_feat[t] @ WU^T)[a]
                + (query @ WQ^T)[a] + bias[a]
    score[t]  = fc_w . tanh(z[t])
    p         = sigmoid(score);  align = p / sum(p)
    context   = align @ value                                        [ENC]

Device-side layout (per core, per batch):
  - value is staged host-side as value^T (bf16, [ENC, T]) so the big
    projection runs with the contraction dim (ENC) on partitions.
  - conv+WU collapse into a rank-3 term: u[t,:] = sum_j W3[:,j]*la[t+j-1],
    where W3 = WU @ conv_w — computed on device from WU^T and conv_w.
  - z tiles are [a_chunk(128), t(512)] psum banks: 1 K=3 matmul (u term)
    + 8 K=128 matmuls (value^T @ WV^T chunks); per-batch constant
    q + bias + WU@conv_b enters via the tanh activation's per-partition bias.
  - score: 4 fc matmuls (M=1) accumulate into psum [1, 512]; sigmoid (ACT)
    with accum_out produces the per-tile sum of p for free.
  - context: p broadcast to 128 partitions via a K=1 ones-matmul, then one
    fused DVE tensor_tensor_reduce per e-chunk does mul+row-reduce+chain-
    accumulate into ctx_acc[128, 8].
"""

import os
import sys
import functools

_TRN_REPO = "/opt/trn_rl_repo"
if _TRN_REPO not in sys.path and os.path.isdir(_TRN_REPO):
    sys.path.insert(0, _TRN_REPO)

import numpy as np
import ml_dtypes

BF16 = ml_dtypes.bfloat16

B, T_FULL, DEC, ENC, ATTN, KCONV = 32, 4096, 1024, 1024, 512, 10
N_CORES = 8
BPC = B // N_CORES          # batches per core
TT = 512                    # t-tile (columns per psum bank)
EC = ENC // 128             # e chunks (8)
AC = ATTN // 128            # a chunks (4)


def build_kernel(bpc=BPC, t_len=T_FULL):
    """Build the Bass module for one core handling `bpc` batches of length t_len."""
    import concourse.bass as bass  # noqa: F401
    import concourse.tile as tile
    from concourse import bacc, mybir

    f32 = mybir.dt.float32
    bf16 = mybir.dt.bfloat16
    AF = mybir.ActivationFunctionType
    ALU = mybir.AluOpType

    n_tiles = t_len // TT

    nc = bacc.Bacc(trn_type="TRN2")

    # ---- DRAM I/O ----
    vT = nc.dram_tensor("vT", [bpc, ENC, t_len], bf16, kind="ExternalInput")
    la3 = nc.dram_tensor("la3", [bpc, 3, t_len], bf16, kind="ExternalInput")
    wvt = nc.dram_tensor("wvt", [ENC, ATTN], bf16, kind="ExternalInput")
    wqt = nc.dram_tensor("wqt", [DEC, ATTN], bf16, kind="ExternalInput")
    qT = nc.dram_tensor("qT", [DEC, bpc], bf16, kind="ExternalInput")
    wut = nc.dram_tensor("wut", [KCONV, ATTN], bf16, kind="ExternalInput")
    cw = nc.dram_tensor("cw", [KCONV, 3], bf16, kind="ExternalInput")
    cb = nc.dram_tensor("cb", [KCONV, bpc], bf16, kind="ExternalInput")
    biasT = nc.dram_tensor("biasT", [128, AC], f32, kind="ExternalInput")
    fcr = nc.dram_tensor("fcr", [128, AC], bf16, kind="ExternalInput")

    ctx_out = nc.dram_tensor("ctx_out", [bpc, ENC], f32, kind="ExternalOutput")
    align_out = nc.dram_tensor("align_out", [bpc, t_len], f32, kind="ExternalOutput")

    with tile.TileContext(nc) as tc:
        with (
            tc.tile_pool(name="consts", bufs=1) as consts,
            tc.tile_pool(name="setupp", bufs=1) as setupp,
            tc.tile_pool(name="vtp", bufs=6) as vtp,
            tc.tile_pool(name="la3p", bufs=2) as la3p,
            tc.tile_pool(name="tanhp", bufs=20) as tanhp,
            tc.tile_pool(name="pbp", bufs=3) as pbp,
            tc.tile_pool(name="scrp", bufs=2) as scrp,
            tc.tile_pool(name="batchp", bufs=2) as batchp,
            tc.tile_pool(name="dramp", bufs=2, space="DRAM") as dramp,
            tc.tile_pool(name="psz", bufs=6, space="PSUM") as psz,
            tc.tile_pool(name="pss", bufs=1, space="PSUM") as pss,
            tc.tile_pool(name="psmisc", bufs=1, space="PSUM") as psmisc,
        ):
            # ---- constant loads ----
            wvt_sb = consts.tile([128, EC, ATTN], bf16)
            nc.sync.dma_start(wvt_sb, wvt.rearrange("(c p) a -> p c a", p=128))
            wqt_sb = setupp.tile([128, DEC // 128, ATTN], bf16)
            nc.sync.dma_start(wqt_sb, wqt.rearrange("(c p) a -> p c a", p=128))
            qT_sb = setupp.tile([128, DEC // 128, bpc], bf16)
            nc.sync.dma_start(qT_sb, qT.rearrange("(c p) b -> p c b", p=128))
            wut_sb = consts.tile([KCONV, ATTN], bf16)
            nc.sync.dma_start(wut_sb, wut[:])
            cw_sb = consts.tile([KCONV, 3], bf16)
            nc.sync.dma_start(cw_sb, cw[:])
            cb_sb = consts.tile([KCONV, bpc], bf16)
            nc.sync.dma_start(cb_sb, cb[:])
            biasT_sb = consts.tile([128, AC], f32)
            nc.sync.dma_start(biasT_sb, biasT[:])
            fcr_sb = consts.tile([128, AC], bf16)
            nc.sync.dma_start(fcr_sb, fcr[:])

            ones_sb = consts.tile([1, 128], f32)
            nc.vector.memset(ones_sb, 1.0)

            # ---- W3T = cw^T @ WU^T : [3, ATTN] ----
            w3_ps = psmisc.tile([3, ATTN], f32, tag="mps")
            nc.tensor.matmul(w3_ps, lhsT=cw_sb, rhs=wut_sb, start=True, stop=True)
            w3_sb = consts.tile([3, ATTN], bf16)
            nc.scalar.copy(w3_sb, w3_ps)

            # ---- cT[a, chunk, b] = (query @ WQ^T + WU @ conv_b + bias)^T ----
            cT_sb = consts.tile([128, AC, bpc], f32)
            for a in range(AC):
                qt_ps = psmisc.tile([128, bpc], f32, tag="mps")
                for c in range(DEC // 128):
                    nc.tensor.matmul(
                        qt_ps,
                        lhsT=wqt_sb[:, c, a * 128:(a + 1) * 128],
                        rhs=qT_sb[:, c, :],
                        start=(c == 0),
                        stop=False,
                    )
                nc.tensor.matmul(
                    qt_ps,
                    lhsT=wut_sb[:, a * 128:(a + 1) * 128],
                    rhs=cb_sb,
                    start=False,
                    stop=True,
                )
                nc.scalar.activation(
                    cT_sb[:, a, :], qt_ps, AF.Identity,
                    bias=biasT_sb[:, a:a + 1], scale=1.0,
                )

            # ---- main loops ----
            HT = min(1024, t_len)       # columns per vT DMA tile
            spt = HT // TT              # subtiles per DMA tile
            n_ht = t_len // HT
            for b in range(bpc):
                la3_sb = la3p.tile([3, t_len], bf16)
                nc.sync.dma_start(la3_sb, la3[b])

                p_row = batchp.tile([1, t_len], f32, tag="p_row")
                p_dram = dramp.tile([t_len], f32, tag="p_dram")
                psums = batchp.tile([1, n_tiles], f32, tag="psums")
                parts = batchp.tile([128, EC, n_tiles], f32, tag="parts")

                ht_tiles = {}
                for h in range(n_ht):
                    vt_tile = vtp.tile([128, EC, HT], bf16, tag="vt",
                                       name=f"vt_{b}_{h}")
                    nc.sync.dma_start(
                        vt_tile,
                        vT[b].rearrange("(c p) t -> p c t", p=128)
                        [:, :, h * HT:(h + 1) * HT],
                    )
                    ht_tiles[h] = vt_tile

                def vt_sl(ti):
                    """(tile, free-slice) of subtile ti's columns."""
                    h, off = divmod(ti * TT, HT)
                    return ht_tiles[h], slice(off, off + TT)

                # group plan: JJ subtiles share each stationary load; taper
                # the final groups of the last batch to shorten the drain.
                plan = []
                rem = n_tiles
                while rem > 0:
                    jj = min(4, rem)
                    plan.append(jj)
                    rem -= jj
                if b == bpc - 1 and plan and plan[-1] == 4:
                    plan = plan[:-1] + [2, 1, 1]

                s0 = 0
                for gi, jj in enumerate(plan):
                    subt = [s0 + j for j in range(jj)]
                    s0 += jj

                    tanhs = {}
                    for a in range(AC):
                        zs = []
                        for j, ti in enumerate(subt):
                            z_ps = psz.tile([128, TT], f32, tag="z",
                                            name=f"z_{b}_{gi}_{a}_{j}")
                            zs.append(z_ps)
                            nc.tensor.matmul(
                                z_ps,
                                lhsT=w3_sb[:, a * 128:(a + 1) * 128],
                                rhs=la3_sb[:, ti * TT:(ti + 1) * TT],
                                start=True,
                                stop=False,
                            )
                        for c in range(EC):
                            for j, ti in enumerate(subt):
                                vt_t, fsl = vt_sl(ti)
                                nc.tensor.matmul(
                                    zs[j],
                                    lhsT=wvt_sb[:, c, a * 128:(a + 1) * 128],
                                    rhs=vt_t[:, c, fsl],
                                    start=False,
                                    stop=(c == EC - 1),
                                )
                        for j, ti in enumerate(subt):
                            tanh_sb = tanhp.tile([128, TT], bf16, tag="tanh",
                                                 name=f"tanh_{b}_{gi}_{a}_{j}")
                            nc.scalar.activation(
                                tanh_sb, zs[j], AF.Tanh,
                                bias=cT_sb[:, a, b:b + 1], scale=1.0,
                            )
                            tanhs[(a, j)] = tanh_sb

                    for j, ti in enumerate(subt):
                        tsl = slice(ti * TT, (ti + 1) * TT)
                        s_ps = pss.tile([1, TT], f32, tag="s",
                                        name=f"s_{b}_{gi}_{j}")
                        for a in range(AC):
                            nc.tensor.matmul(
                                s_ps,
                                lhsT=fcr_sb[:, a:a + 1],
                                rhs=tanhs[(a, j)],
                                start=(a == 0),
                                stop=(a == AC - 1),
                            )

                        nc.scalar.activation(
                            p_row[0:1, tsl], s_ps, AF.Sigmoid,
                            accum_out=psums[0:1, ti:ti + 1],
                        )

                        nc.scalar.dma_start(
                            p_dram[tsl][None, :], p_row[0:1, tsl]
                        )
                        pb_sb = pbp.tile([128, TT], bf16, tag="pb_sb",
                                         name=f"pb_{b}_{gi}_{j}")
                        nc.gpsimd.dma_start(
                            pb_sb,
                            p_dram[tsl][None, :].to_broadcast([128, TT]),
                        )

                        scr = scrp.tile([128, TT], bf16, tag="scr",
                                        name=f"scr_{b}_{gi}_{j}")
                        vt_t, fsl = vt_sl(ti)
                        for c in range(EC):
                            nc.vector.scalar_tensor_tensor(
                                out=scr,
                                in0=vt_t[:, c, fsl],
                                scalar=1.0,
                                in1=pb_sb,
                                op0=ALU.mult,
                                op1=ALU.mult,
                                accum_out=parts[:, c, ti:ti + 1],
                            )

                # ---- batch epilogue ----
                sum1 = batchp.tile([1, 1], f32, tag="sum1")
                nc.vector.reduce_sum(sum1, psums, axis=mybir.AxisListType.X)
                inv1 = batchp.tile([1, 1], f32, tag="inv1")
                nc.vector.reciprocal(inv1, sum1)

                nc.scalar.activation(
                    p_row, p_row, AF.Copy, scale=inv1[0:1, 0:1]
                )
                nc.scalar.dma_start(align_out[b:b + 1, :], p_row)

                invp_ps = psmisc.tile([128, 1], f32, tag="mps",
                                      name=f"invp_{b}")
                nc.tensor.matmul(invp_ps, lhsT=ones_sb, rhs=inv1, start=True, stop=True)
                invp_sb = batchp.tile([128, 1], f32, tag="invp_sb")
                nc.scalar.copy(invp_sb, invp_ps)

                ctx_acc = batchp.tile([128, EC], f32, tag="ctx_acc")
                for c in range(EC):
                    nc.vector.reduce_sum(
                        ctx_acc[:, c:c + 1], parts[:, c, :],
                        axis=mybir.AxisListType.X,
                    )
                ctx_sb = batchp.tile([128, EC], f32, tag="ctx_sb")
                nc.vector.tensor_scalar_mul(ctx_sb, ctx_acc, invp_sb)
                nc.scalar.dma_start(
                    ctx_out[b].rearrange("(c p) -> p c", p=128), ctx_sb
                )

    nc.compile()
    return nc


def prep_inputs(query, value, last_align, conv_w, conv_b, WQ, WV, WU, bias, fc_w,
                bpc=BPC, n_cores=N_CORES):
    """Host-side sharding + layout prep. Returns list of per-core input dicts."""
    t_len = value.shape[1]

    value = np.asarray(value, np.float32)
    la = np.asarray(last_align, np.float32)

    # value^T per batch, bf16
    vT_all = np.ascontiguousarray(value.transpose(0, 2, 1)).astype(BF16)

    nb = vT_all.shape[0]
    la3_all = np.zeros((nb, 3, t_len), np.float32)
    la3_all[:, 0, 1:] = la[:, :-1]
    la3_all[:, 1, :] = la
    la3_all[:, 2, :-1] = la[:, 1:]
    la3_all = la3_all.astype(BF16)

    wvt_h = np.ascontiguousarray(np.asarray(WV, np.float32).T).astype(BF16)
    wqt_h = np.ascontiguousarray(np.asarray(WQ, np.float32).T).astype(BF16)
    wut_h = np.ascontiguousarray(np.asarray(WU, np.float32).T).astype(BF16)
    cw_h = np.ascontiguousarray(np.asarray(conv_w, np.float32)[:, 0, :]).astype(BF16)
    biasT_h = np.ascontiguousarray(
        np.asarray(bias, np.float32).reshape(AC, 128).T
    ).astype(np.float32)
    fcr_h = np.ascontiguousarray(
        np.asarray(fc_w, np.float32)[0].reshape(AC, 128).T
    ).astype(BF16)

    qT_all = np.ascontiguousarray(
        np.asarray(query, np.float32)[:, 0, :].T
    ).astype(BF16)  # [DEC, B]
    cb_h = np.ascontiguousarray(
        np.repeat(np.asarray(conv_b, np.float32)[:, None], bpc, axis=1)
    ).astype(BF16)

    in_maps = []
    for c in range(n_cores):
        bs = slice(c * bpc, (c + 1) * bpc)
        in_maps.append({
            "vT": np.ascontiguousarray(vT_all[bs]),
            "la3": np.ascontiguousarray(la3_all[bs]),
            "wvt": wvt_h,
            "wqt": wqt_h,
            "qT": np.ascontiguousarray(qT_all[:, bs]),
            "wut": wut_h,
            "cw": cw_h,
            "cb": cb_h,
            "biasT": biasT_h,
            "fcr": fcr_h,
        })
    return in_maps


@functools.lru_cache(maxsize=1)
def _get_nc():
    return build_kernel()


def run(inputs, trace=False, **kwargs):
    from concourse.bass_utils import run_bass_kernel_spmd

    nc = _get_nc()
    in_maps = prep_inputs(**inputs)
    res = run_bass_kernel_spmd(
        nc, in_maps, core_ids=list(range(N_CORES)), trace=trace, **kwargs
    )
    ctx = np.concatenate([np.asarray(r["ctx_out"]) for r in res.results], axis=0)
    align = np.concatenate([np.asarray(r["align_out"]) for r in res.results], axis=0)
    return (ctx.astype(np.float32), align.astype(np.float32)), res


def kernel(**inputs):
    (ctx, align), _ = run(inputs, trace=False)
    return ctx, align


# revision 17
# speedup vs baseline: 1.2927x; 1.0254x over previous
"""LocationAwareAttention Trainium2 kernel.

Full-input contract: kernel(**inputs) takes the complete unsharded inputs
(as produced by the problem's setup_inputs) and returns (context, align) as
full-shape fp32 arrays.  Internally the batch dimension (B=32) is sharded
across 8 NeuronCores (4 batches per core); all weights are replicated.

Math (per batch b):
    conv_feat = conv1d(last_align, conv_w, pad=1) + conv_b          [T, K]
    z[t, a]   = (value[t] @ WV^T)[a] + (conv_feat[t] @ WU^T)[a]
                + (query @ WQ^T)[a] + bias[a]
    score[t]  = fc_w . tanh(z[t])
    p         = sigmoid(score);  align = p / sum(p)
    context   = align @ value                                        [ENC]

Device-side layout (per core, per batch):
  - value is staged host-side as value^T (bf16, [ENC, T]) so the big
    projection runs with the contraction dim (ENC) on partitions.
  - conv+WU collapse into a rank-3 term: u[t,:] = sum_j W3[:,j]*la[t+j-1],
    where W3 = WU @ conv_w — computed on device from WU^T and conv_w.
  - z tiles are [a_chunk(128), t(512)] psum banks: 1 K=3 matmul (u term)
    + 8 K=128 matmuls (value^T @ WV^T chunks); per-batch constant
    q + bias + WU@conv_b enters via the tanh activation's per-partition bias.
  - score: 4 fc matmuls (M=1) accumulate into psum [1, 512]; sigmoid (ACT)
    with accum_out produces the per-tile sum of p for free.
  - context: p broadcast to 128 partitions via a K=1 ones-matmul, then one
    fused DVE tensor_tensor_reduce per e-chunk does mul+row-reduce+chain-
    accumulate into ctx_acc[128, 8].
"""

import os
import sys
import functools

_TRN_REPO = "/opt/trn_rl_repo"
if _TRN_REPO not in sys.path and os.path.isdir(_TRN_REPO):
    sys.path.insert(0, _TRN_REPO)

import numpy as np
import ml_dtypes

BF16 = ml_dtypes.bfloat16

B, T_FULL, DEC, ENC, ATTN, KCONV = 32, 4096, 1024, 1024, 512, 10
N_CORES = 8
BPC = B // N_CORES          # batches per core
TT = 512                    # t-tile (columns per psum bank)
EC = ENC // 128             # e chunks (8)
AC = ATTN // 128            # a chunks (4)


def build_kernel(bpc=BPC, t_len=T_FULL):
    """Build the Bass module for one core handling `bpc` batches of length t_len."""
    import concourse.bass as bass  # noqa: F401
    import concourse.tile as tile
    from concourse import bacc, mybir

    f32 = mybir.dt.float32
    bf16 = mybir.dt.bfloat16
    AF = mybir.ActivationFunctionType
    ALU = mybir.AluOpType

    n_tiles = t_len // TT

    nc = bacc.Bacc(trn_type="TRN2")

    # ---- DRAM I/O ----
    vT = nc.dram_tensor("vT", [bpc, ENC, t_len], bf16, kind="ExternalInput")
    la3 = nc.dram_tensor("la3", [bpc, 3, t_len], bf16, kind="ExternalInput")
    wvt = nc.dram_tensor("wvt", [ENC, ATTN], bf16, kind="ExternalInput")
    wqt = nc.dram_tensor("wqt", [DEC, ATTN], bf16, kind="ExternalInput")
    qT = nc.dram_tensor("qT", [DEC, bpc], bf16, kind="ExternalInput")
    wut = nc.dram_tensor("wut", [KCONV, ATTN], bf16, kind="ExternalInput")
    cw = nc.dram_tensor("cw", [KCONV, 3], bf16, kind="ExternalInput")
    cb = nc.dram_tensor("cb", [KCONV, bpc], bf16, kind="ExternalInput")
    biasT = nc.dram_tensor("biasT", [128, AC], f32, kind="ExternalInput")
    fcr = nc.dram_tensor("fcr", [128, AC], bf16, kind="ExternalInput")

    ctx_out = nc.dram_tensor("ctx_out", [bpc, ENC], f32, kind="ExternalOutput")
    align_out = nc.dram_tensor("align_out", [bpc, t_len], f32, kind="ExternalOutput")

    with tile.TileContext(nc) as tc:
        with (
            tc.tile_pool(name="consts", bufs=1) as consts,
            tc.tile_pool(name="setupp", bufs=1) as setupp,
            tc.tile_pool(name="vtp", bufs=6) as vtp,
            tc.tile_pool(name="la3p", bufs=2) as la3p,
            tc.tile_pool(name="tanhp", bufs=20) as tanhp,
            tc.tile_pool(name="pbp", bufs=3) as pbp,
            tc.tile_pool(name="scrp", bufs=2) as scrp,
            tc.tile_pool(name="batchp", bufs=2) as batchp,
            tc.tile_pool(name="dramp", bufs=2, space="DRAM") as dramp,
            tc.tile_pool(name="psz", bufs=6, space="PSUM") as psz,
            tc.tile_pool(name="pss", bufs=1, space="PSUM") as pss,
            tc.tile_pool(name="psmisc", bufs=1, space="PSUM") as psmisc,
        ):
            # ---- constant loads ----
            wvt_sb = consts.tile([128, EC, ATTN], bf16)
            nc.sync.dma_start(wvt_sb, wvt.rearrange("(c p) a -> p c a", p=128))
            wqt_sb = setupp.tile([128, DEC // 128, ATTN], bf16)
            nc.scalar.dma_start(wqt_sb, wqt.rearrange("(c p) a -> p c a", p=128))
            qT_sb = setupp.tile([128, DEC // 128, bpc], bf16)
            nc.scalar.dma_start(qT_sb, qT.rearrange("(c p) b -> p c b", p=128))
            wut_sb = consts.tile([KCONV, ATTN], bf16)
            nc.scalar.dma_start(wut_sb, wut[:])
            cw_sb = consts.tile([KCONV, 3], bf16)
            nc.scalar.dma_start(cw_sb, cw[:])
            cb_sb = consts.tile([KCONV, bpc], bf16)
            nc.scalar.dma_start(cb_sb, cb[:])
            biasT_sb = consts.tile([128, AC], f32)
            nc.scalar.dma_start(biasT_sb, biasT[:])
            fcr_sb = consts.tile([128, AC], bf16)
            nc.scalar.dma_start(fcr_sb, fcr[:])

            ones_sb = consts.tile([1, 128], f32)
            nc.vector.memset(ones_sb, 1.0)

            # ---- W3T = cw^T @ WU^T : [3, ATTN] ----
            w3_ps = psmisc.tile([3, ATTN], f32, tag="mps")
            nc.tensor.matmul(w3_ps, lhsT=cw_sb, rhs=wut_sb, start=True, stop=True)
            w3_sb = consts.tile([3, ATTN], bf16)
            nc.scalar.copy(w3_sb, w3_ps)

            # ---- cT[a, chunk, b] = (query @ WQ^T + WU @ conv_b + bias)^T ----
            cT_sb = consts.tile([128, AC, bpc], f32)
            for a in range(AC):
                qt_ps = psmisc.tile([128, bpc], f32, tag="mps")
                for c in range(DEC // 128):
                    nc.tensor.matmul(
                        qt_ps,
                        lhsT=wqt_sb[:, c, a * 128:(a + 1) * 128],
                        rhs=qT_sb[:, c, :],
                        start=(c == 0),
                        stop=False,
                    )
                nc.tensor.matmul(
                    qt_ps,
                    lhsT=wut_sb[:, a * 128:(a + 1) * 128],
                    rhs=cb_sb,
                    start=False,
                    stop=True,
                )
                nc.scalar.activation(
                    cT_sb[:, a, :], qt_ps, AF.Identity,
                    bias=biasT_sb[:, a:a + 1], scale=1.0,
                )

            # ---- main loops ----
            HT = min(1024, t_len)       # columns per vT DMA tile
            spt = HT // TT              # subtiles per DMA tile
            n_ht = t_len // HT
            for b in range(bpc):
                la3_sb = la3p.tile([3, t_len], bf16)
                nc.sync.dma_start(la3_sb, la3[b])

                p_row = batchp.tile([1, t_len], f32, tag="p_row")
                p_dram = dramp.tile([t_len], f32, tag="p_dram")
                psums = batchp.tile([1, n_tiles], f32, tag="psums")
                parts = batchp.tile([128, EC, n_tiles], f32, tag="parts")

                ht_tiles = {}
                for h in range(n_ht):
                    vt_tile = vtp.tile([128, EC, HT], bf16, tag="vt",
                                       name=f"vt_{b}_{h}")
                    nc.sync.dma_start(
                        vt_tile,
                        vT[b].rearrange("(c p) t -> p c t", p=128)
                        [:, :, h * HT:(h + 1) * HT],
                    )
                    ht_tiles[h] = vt_tile

                def vt_sl(ti):
                    """(tile, free-slice) of subtile ti's columns."""
                    h, off = divmod(ti * TT, HT)
                    return ht_tiles[h], slice(off, off + TT)

                # group plan: JJ subtiles share each stationary load; taper
                # the final groups of the last batch to shorten the drain.
                plan = []
                rem = n_tiles
                while rem > 0:
                    jj = min(4, rem)
                    plan.append(jj)
                    rem -= jj
                if b == bpc - 1 and plan and plan[-1] == 4:
                    plan = plan[:-1] + [2, 1, 1]

                s0 = 0
                for gi, jj in enumerate(plan):
                    subt = [s0 + j for j in range(jj)]
                    s0 += jj

                    tanhs = {}
                    for a in range(AC):
                        zs = []
                        for j, ti in enumerate(subt):
                            z_ps = psz.tile([128, TT], f32, tag="z",
                                            name=f"z_{b}_{gi}_{a}_{j}")
                            zs.append(z_ps)
                            nc.tensor.matmul(
                                z_ps,
                                lhsT=w3_sb[:, a * 128:(a + 1) * 128],
                                rhs=la3_sb[:, ti * TT:(ti + 1) * TT],
                                start=True,
                                stop=False,
                            )
                        for c in range(EC):
                            for j, ti in enumerate(subt):
                                vt_t, fsl = vt_sl(ti)
                                nc.tensor.matmul(
                                    zs[j],
                                    lhsT=wvt_sb[:, c, a * 128:(a + 1) * 128],
                                    rhs=vt_t[:, c, fsl],
                                    start=False,
                                    stop=(c == EC - 1),
                                )
                        for j, ti in enumerate(subt):
                            tanh_sb = tanhp.tile([128, TT], bf16, tag="tanh",
                                                 name=f"tanh_{b}_{gi}_{a}_{j}")
                            nc.scalar.activation(
                                tanh_sb, zs[j], AF.Tanh,
                                bias=cT_sb[:, a, b:b + 1], scale=1.0,
                            )
                            tanhs[(a, j)] = tanh_sb

                    for j, ti in enumerate(subt):
                        tsl = slice(ti * TT, (ti + 1) * TT)
                        s_ps = pss.tile([1, TT], f32, tag="s",
                                        name=f"s_{b}_{gi}_{j}")
                        for a in range(AC):
                            nc.tensor.matmul(
                                s_ps,
                                lhsT=fcr_sb[:, a:a + 1],
                                rhs=tanhs[(a, j)],
                                start=(a == 0),
                                stop=(a == AC - 1),
                            )

                        nc.scalar.activation(
                            p_row[0:1, tsl], s_ps, AF.Sigmoid,
                            accum_out=psums[0:1, ti:ti + 1],
                        )

                        nc.scalar.dma_start(
                            p_dram[tsl][None, :], p_row[0:1, tsl]
                        )
                        pb_sb = pbp.tile([128, TT], bf16, tag="pb_sb",
                                         name=f"pb_{b}_{gi}_{j}")
                        nc.gpsimd.dma_start(
                            pb_sb,
                            p_dram[tsl][None, :].to_broadcast([128, TT]),
                        )

                        scr = scrp.tile([128, TT], bf16, tag="scr",
                                        name=f"scr_{b}_{gi}_{j}")
                        vt_t, fsl = vt_sl(ti)
                        for c in range(EC):
                            nc.vector.scalar_tensor_tensor(
                                out=scr,
                                in0=vt_t[:, c, fsl],
                                scalar=1.0,
                                in1=pb_sb,
                                op0=ALU.mult,
                                op1=ALU.mult,
                                accum_out=parts[:, c, ti:ti + 1],
                            )

                # ---- batch epilogue ----
                sum1 = batchp.tile([1, 1], f32, tag="sum1")
                nc.vector.reduce_sum(sum1, psums, axis=mybir.AxisListType.X)
                inv1 = batchp.tile([1, 1], f32, tag="inv1")
                nc.vector.reciprocal(inv1, sum1)

                nc.vector.tensor_scalar_mul(p_row, p_row, inv1)
                nc.gpsimd.dma_start(align_out[b:b + 1, :], p_row)

                i_dram = dramp.tile([1], f32, tag="i_dram")
                nc.gpsimd.dma_start(i_dram[None, :], inv1)
                invp_sb = batchp.tile([128, 1], f32, tag="invp_sb")
                nc.gpsimd.dma_start(
                    invp_sb, i_dram[None, :].to_broadcast([128, 1])
                )

                ctx_acc = batchp.tile([128, EC], f32, tag="ctx_acc")
                for c in range(EC):
                    nc.vector.reduce_sum(
                        ctx_acc[:, c:c + 1], parts[:, c, :],
                        axis=mybir.AxisListType.X,
                    )
                ctx_sb = batchp.tile([128, EC], f32, tag="ctx_sb")
                nc.vector.tensor_scalar_mul(ctx_sb, ctx_acc, invp_sb)
                nc.gpsimd.dma_start(
                    ctx_out[b].rearrange("(c p) -> p c", p=128), ctx_sb
                )

    nc.compile()
    return nc


def prep_inputs(query, value, last_align, conv_w, conv_b, WQ, WV, WU, bias, fc_w,
                bpc=BPC, n_cores=N_CORES):
    """Host-side sharding + layout prep. Returns list of per-core input dicts."""
    t_len = value.shape[1]

    value = np.asarray(value, np.float32)
    la = np.asarray(last_align, np.float32)

    # value^T per batch, bf16
    vT_all = np.ascontiguousarray(value.transpose(0, 2, 1)).astype(BF16)

    nb = vT_all.shape[0]
    la3_all = np.zeros((nb, 3, t_len), np.float32)
    la3_all[:, 0, 1:] = la[:, :-1]
    la3_all[:, 1, :] = la
    la3_all[:, 2, :-1] = la[:, 1:]
    la3_all = la3_all.astype(BF16)

    wvt_h = np.ascontiguousarray(np.asarray(WV, np.float32).T).astype(BF16)
    wqt_h = np.ascontiguousarray(np.asarray(WQ, np.float32).T).astype(BF16)
    wut_h = np.ascontiguousarray(np.asarray(WU, np.float32).T).astype(BF16)
    cw_h = np.ascontiguousarray(np.asarray(conv_w, np.float32)[:, 0, :]).astype(BF16)
    biasT_h = np.ascontiguousarray(
        np.asarray(bias, np.float32).reshape(AC, 128).T
    ).astype(np.float32)
    fcr_h = np.ascontiguousarray(
        np.asarray(fc_w, np.float32)[0].reshape(AC, 128).T
    ).astype(BF16)

    qT_all = np.ascontiguousarray(
        np.asarray(query, np.float32)[:, 0, :].T
    ).astype(BF16)  # [DEC, B]
    cb_h = np.ascontiguousarray(
        np.repeat(np.asarray(conv_b, np.float32)[:, None], bpc, axis=1)
    ).astype(BF16)

    in_maps = []
    for c in range(n_cores):
        bs = slice(c * bpc, (c + 1) * bpc)
        in_maps.append({
            "vT": np.ascontiguousarray(vT_all[bs]),
            "la3": np.ascontiguousarray(la3_all[bs]),
            "wvt": wvt_h,
            "wqt": wqt_h,
            "qT": np.ascontiguousarray(qT_all[:, bs]),
            "wut": wut_h,
            "cw": cw_h,
            "cb": cb_h,
            "biasT": biasT_h,
            "fcr": fcr_h,
        })
    return in_maps


@functools.lru_cache(maxsize=1)
def _get_nc():
    return build_kernel()


def run(inputs, trace=False, **kwargs):
    from concourse.bass_utils import run_bass_kernel_spmd

    nc = _get_nc()
    in_maps = prep_inputs(**inputs)
    res = run_bass_kernel_spmd(
        nc, in_maps, core_ids=list(range(N_CORES)), trace=trace, **kwargs
    )
    ctx = np.concatenate([np.asarray(r["ctx_out"]) for r in res.results], axis=0)
    align = np.concatenate([np.asarray(r["align_out"]) for r in res.results], axis=0)
    return (ctx.astype(np.float32), align.astype(np.float32)), res


def kernel(**inputs):
    (ctx, align), _ = run(inputs, trace=False)
    return ctx, align


# revision 19
# speedup vs baseline: 1.3140x; 1.0164x over previous
"""LocationAwareAttention Trainium2 kernel.

Full-input contract: kernel(**inputs) takes the complete unsharded inputs
(as produced by the problem's setup_inputs) and returns (context, align) as
full-shape fp32 arrays.  Internally the batch dimension (B=32) is sharded
across 8 NeuronCores (4 batches per core); all weights are replicated.

Math (per batch b):
    conv_feat = conv1d(last_align, conv_w, pad=1) + conv_b          [T, K]
    z[t, a]   = (value[t] @ WV^T)[a] + (conv_feat[t] @ WU^T)[a]
                + (query @ WQ^T)[a] + bias[a]
    score[t]  = fc_w . tanh(z[t])
    p         = sigmoid(score);  align = p / sum(p)
    context   = align @ value                                        [ENC]

Device-side layout (per core, per batch):
  - value is staged host-side as value^T (bf16, [ENC, T]) so the big
    projection runs with the contraction dim (ENC) on partitions.
  - conv+WU collapse into a rank-3 term: u[t,:] = sum_j W3[:,j]*la[t+j-1],
    where W3 = WU @ conv_w — computed on device from WU^T and conv_w.
  - z tiles are [a_chunk(128), t(512)] psum banks: 1 K=3 matmul (u term)
    + 8 K=128 matmuls (value^T @ WV^T chunks); per-batch constant
    q + bias + WU@conv_b enters via the tanh activation's per-partition bias.
  - score: 4 fc matmuls (M=1) accumulate into psum [1, 512]; sigmoid (ACT)
    with accum_out produces the per-tile sum of p for free.
  - context: p broadcast to 128 partitions via a K=1 ones-matmul, then one
    fused DVE tensor_tensor_reduce per e-chunk does mul+row-reduce+chain-
    accumulate into ctx_acc[128, 8].
"""

import os
import sys
import functools

_TRN_REPO = "/opt/trn_rl_repo"
if _TRN_REPO not in sys.path and os.path.isdir(_TRN_REPO):
    sys.path.insert(0, _TRN_REPO)

import numpy as np
import ml_dtypes

BF16 = ml_dtypes.bfloat16

B, T_FULL, DEC, ENC, ATTN, KCONV = 32, 4096, 1024, 1024, 512, 10
N_CORES = 8
BPC = B // N_CORES          # batches per core
TT = 512                    # t-tile (columns per psum bank)
EC = ENC // 128             # e chunks (8)
AC = ATTN // 128            # a chunks (4)


def build_kernel(bpc=BPC, t_len=T_FULL):
    """Build the Bass module for one core handling `bpc` batches of length t_len."""
    import concourse.bass as bass  # noqa: F401
    import concourse.tile as tile
    from concourse import bacc, mybir

    f32 = mybir.dt.float32
    bf16 = mybir.dt.bfloat16
    AF = mybir.ActivationFunctionType
    ALU = mybir.AluOpType

    n_tiles = t_len // TT

    nc = bacc.Bacc(trn_type="TRN2")

    # ---- DRAM I/O ----
    vT = nc.dram_tensor("vT", [bpc, ENC, t_len], bf16, kind="ExternalInput")
    la3 = nc.dram_tensor("la3", [bpc, 3, t_len], bf16, kind="ExternalInput")
    wvt = nc.dram_tensor("wvt", [ENC, ATTN], bf16, kind="ExternalInput")
    wqt = nc.dram_tensor("wqt", [DEC, ATTN], bf16, kind="ExternalInput")
    qT = nc.dram_tensor("qT", [DEC, bpc], bf16, kind="ExternalInput")
    wut = nc.dram_tensor("wut", [KCONV, ATTN], bf16, kind="ExternalInput")
    cw = nc.dram_tensor("cw", [KCONV, 3], bf16, kind="ExternalInput")
    cb = nc.dram_tensor("cb", [KCONV, bpc], bf16, kind="ExternalInput")
    biasT = nc.dram_tensor("biasT", [128, AC], f32, kind="ExternalInput")
    fcr = nc.dram_tensor("fcr", [128, AC], bf16, kind="ExternalInput")

    ctx_out = nc.dram_tensor("ctx_out", [bpc, ENC], f32, kind="ExternalOutput")
    align_out = nc.dram_tensor("align_out", [bpc, t_len], f32, kind="ExternalOutput")

    with tile.TileContext(nc) as tc:
        with (
            tc.tile_pool(name="consts", bufs=1) as consts,
            tc.tile_pool(name="setupp", bufs=1) as setupp,
            tc.tile_pool(name="vtp", bufs=6) as vtp,
            tc.tile_pool(name="la3p", bufs=2) as la3p,
            tc.tile_pool(name="tanhp", bufs=20) as tanhp,
            tc.tile_pool(name="pbp", bufs=3) as pbp,
            tc.tile_pool(name="scrp", bufs=2) as scrp,
            tc.tile_pool(name="batchp", bufs=2) as batchp,
            tc.tile_pool(name="dramp", bufs=2, space="DRAM") as dramp,
            tc.tile_pool(name="psz", bufs=6, space="PSUM") as psz,
            tc.tile_pool(name="pss", bufs=1, space="PSUM") as pss,
            tc.tile_pool(name="psmisc", bufs=1, space="PSUM") as psmisc,
        ):
            # ---- constant loads ----
            wvt_sb = consts.tile([128, EC, ATTN], bf16)
            nc.gpsimd.dma_start(wvt_sb, wvt.rearrange("(c p) a -> p c a", p=128))
            wqt_sb = setupp.tile([128, DEC // 128, ATTN], bf16)
            nc.scalar.dma_start(wqt_sb, wqt.rearrange("(c p) a -> p c a", p=128))
            qT_sb = setupp.tile([128, DEC // 128, bpc], bf16)
            nc.scalar.dma_start(qT_sb, qT.rearrange("(c p) b -> p c b", p=128))
            wut_sb = consts.tile([KCONV, ATTN], bf16)
            nc.scalar.dma_start(wut_sb, wut[:])
            cw_sb = consts.tile([KCONV, 3], bf16)
            nc.scalar.dma_start(cw_sb, cw[:])
            cb_sb = consts.tile([KCONV, bpc], bf16)
            nc.scalar.dma_start(cb_sb, cb[:])
            biasT_sb = consts.tile([128, AC], f32)
            nc.scalar.dma_start(biasT_sb, biasT[:])
            fcr_sb = consts.tile([128, AC], bf16)
            nc.scalar.dma_start(fcr_sb, fcr[:])

            ones_sb = consts.tile([1, 128], f32)
            nc.vector.memset(ones_sb, 1.0)
            ident_sb = consts.tile([128, 128], f32)
            from concourse.masks import make_identity
            make_identity(nc, ident_sb)

            # ---- W3T = cw^T @ WU^T : [3, ATTN] ----
            w3_ps = psmisc.tile([3, ATTN], f32, tag="mps")
            nc.tensor.matmul(w3_ps, lhsT=cw_sb, rhs=wut_sb, start=True, stop=True)
            w3_sb = consts.tile([3, ATTN], bf16)
            nc.scalar.copy(w3_sb, w3_ps)

            # ---- cT[a, chunk, b] = (query @ WQ^T + WU @ conv_b + bias)^T ----
            cT_sb = consts.tile([128, AC, bpc], f32)
            for a in range(AC):
                qt_ps = psmisc.tile([128, bpc], f32, tag="mps")
                for c in range(DEC // 128):
                    nc.tensor.matmul(
                        qt_ps,
                        lhsT=wqt_sb[:, c, a * 128:(a + 1) * 128],
                        rhs=qT_sb[:, c, :],
                        start=(c == 0),
                        stop=False,
                    )
                nc.tensor.matmul(
                    qt_ps,
                    lhsT=wut_sb[:, a * 128:(a + 1) * 128],
                    rhs=cb_sb,
                    start=False,
                    stop=True,
                )
                nc.scalar.activation(
                    cT_sb[:, a, :], qt_ps, AF.Identity,
                    bias=biasT_sb[:, a:a + 1], scale=1.0,
                )

            # ---- main loops ----
            HT = min(1024, t_len)       # columns per vT DMA tile
            spt = HT // TT              # subtiles per DMA tile
            n_ht = t_len // HT
            for b in range(bpc):
                la3_sb = la3p.tile([3, t_len], bf16)
                nc.sync.dma_start(la3_sb, la3[b])

                p_row = batchp.tile([1, t_len], f32, tag="p_row")
                p_dram = dramp.tile([t_len], f32, tag="p_dram")
                psums = batchp.tile([1, n_tiles], f32, tag="psums")
                parts = batchp.tile([128, EC, n_tiles], f32, tag="parts")

                ht_tiles = {}
                vT_r = vT[b].rearrange("(c p) t -> p c t", p=128)
                for h in range(n_ht):
                    vt_tile = vtp.tile([128, EC, HT], bf16, tag="vt",
                                       name=f"vt_{b}_{h}")
                    if b == 0 and h == 0:
                        half = HT // 2
                        nc.sync.dma_start(vt_tile[:, :, :half],
                                          vT_r[:, :, :half])
                        nc.sync.dma_start(vt_tile[:, :, half:HT],
                                          vT_r[:, :, half:HT])
                    else:
                        nc.sync.dma_start(
                            vt_tile, vT_r[:, :, h * HT:(h + 1) * HT]
                        )
                    ht_tiles[h] = vt_tile

                def vt_sl(ti):
                    """(tile, free-slice) of subtile ti's columns."""
                    h, off = divmod(ti * TT, HT)
                    return ht_tiles[h], slice(off, off + TT)

                # group plan: JJ subtiles share each stationary load; taper
                # the final groups of the last batch to shorten the drain.
                plan = []
                rem = n_tiles
                while rem > 0:
                    jj = min(4, rem)
                    plan.append(jj)
                    rem -= jj
                if b == bpc - 1 and plan and plan[-1] == 4:
                    plan = plan[:-1] + [2, 1, 1]

                s0 = 0
                for gi, jj in enumerate(plan):
                    subt = [s0 + j for j in range(jj)]
                    s0 += jj

                    tanhs = {}
                    for a in range(AC):
                        zs = []
                        for j, ti in enumerate(subt):
                            z_ps = psz.tile([128, TT], f32, tag="z",
                                            name=f"z_{b}_{gi}_{a}_{j}")
                            zs.append(z_ps)
                            nc.tensor.matmul(
                                z_ps,
                                lhsT=w3_sb[:, a * 128:(a + 1) * 128],
                                rhs=la3_sb[:, ti * TT:(ti + 1) * TT],
                                start=True,
                                stop=False,
                            )
                        for c in range(EC):
                            for j, ti in enumerate(subt):
                                vt_t, fsl = vt_sl(ti)
                                nc.tensor.matmul(
                                    zs[j],
                                    lhsT=wvt_sb[:, c, a * 128:(a + 1) * 128],
                                    rhs=vt_t[:, c, fsl],
                                    start=False,
                                    stop=(c == EC - 1),
                                )
                        for j, ti in enumerate(subt):
                            tanh_sb = tanhp.tile([128, TT], bf16, tag="tanh",
                                                 name=f"tanh_{b}_{gi}_{a}_{j}")
                            nc.scalar.activation(
                                tanh_sb, zs[j], AF.Tanh,
                                bias=cT_sb[:, a, b:b + 1], scale=1.0,
                            )
                            tanhs[(a, j)] = tanh_sb

                    for j, ti in enumerate(subt):
                        tsl = slice(ti * TT, (ti + 1) * TT)
                        s_ps = pss.tile([1, TT], f32, tag="s",
                                        name=f"s_{b}_{gi}_{j}")
                        for a in range(AC):
                            nc.tensor.matmul(
                                s_ps,
                                lhsT=fcr_sb[:, a:a + 1],
                                rhs=tanhs[(a, j)],
                                start=(a == 0),
                                stop=(a == AC - 1),
                            )

                        nc.scalar.activation(
                            p_row[0:1, tsl], s_ps, AF.Sigmoid,
                            accum_out=psums[0:1, ti:ti + 1],
                        )

                        nc.scalar.dma_start(
                            p_dram[tsl][None, :], p_row[0:1, tsl]
                        )
                        pb_sb = pbp.tile([128, TT], bf16, tag="pb_sb",
                                         name=f"pb_{b}_{gi}_{j}")
                        nc.gpsimd.dma_start(
                            pb_sb,
                            p_dram[tsl][None, :].to_broadcast([128, TT]),
                        )

                        scr = scrp.tile([128, TT], bf16, tag="scr",
                                        name=f"scr_{b}_{gi}_{j}")
                        vt_t, fsl = vt_sl(ti)
                        for c in range(EC):
                            nc.vector.scalar_tensor_tensor(
                                out=scr,
                                in0=vt_t[:, c, fsl],
                                scalar=1.0,
                                in1=pb_sb,
                                op0=ALU.mult,
                                op1=ALU.mult,
                                accum_out=parts[:, c, ti:ti + 1],
                            )

                # ---- batch epilogue ----
                sum1 = batchp.tile([1, 1], f32, tag="sum1")
                nc.vector.reduce_sum(sum1, psums, axis=mybir.AxisListType.X)
                inv1 = batchp.tile([1, 1], f32, tag="inv1")
                nc.vector.reciprocal(inv1, sum1)

                nc.vector.tensor_scalar_mul(p_row, p_row, inv1)
                nc.gpsimd.dma_start(align_out[b:b + 1, :], p_row)

                invp_sb = batchp.tile([128, 1], f32, tag="invp_sb")
                if b == bpc - 1:
                    invp_ps = psmisc.tile([128, 1], f32, tag="mps",
                                          name=f"invp_{b}")
                    nc.tensor.matmul(invp_ps, lhsT=ones_sb, rhs=inv1,
                                     start=True, stop=True)
                    nc.scalar.copy(invp_sb, invp_ps)
                else:
                    i_dram = dramp.tile([1], f32, tag="i_dram")
                    nc.gpsimd.dma_start(i_dram[None, :], inv1)
                    nc.gpsimd.dma_start(
                        invp_sb, i_dram[None, :].to_broadcast([128, 1])
                    )

                ctx_acc = batchp.tile([128, EC], f32, tag="ctx_acc")
                for c in range(EC):
                    nc.vector.reduce_sum(
                        ctx_acc[:, c:c + 1], parts[:, c, :],
                        axis=mybir.AxisListType.X,
                    )
                ctx_sb = batchp.tile([128, EC], f32, tag="ctx_sb")
                nc.vector.tensor_scalar_mul(ctx_sb, ctx_acc, invp_sb)
                if b == bpc - 1:
                    ctxT_ps = psmisc.tile([EC, 128], f32, tag="mps",
                                          name=f"ctxT_{b}")
                    nc.tensor.transpose(ctxT_ps, ctx_sb, ident_sb)
                    ctxT_sb = batchp.tile([EC, 128], f32, tag="ctxT_sb")
                    nc.scalar.copy(ctxT_sb, ctxT_ps)
                    nc.scalar.dma_start(
                        ctx_out[b].rearrange("(c p) -> c p", p=128), ctxT_sb
                    )
                else:
                    nc.gpsimd.dma_start(
                        ctx_out[b].rearrange("(c p) -> p c", p=128), ctx_sb
                    )

    nc.compile()
    return nc


def prep_inputs(query, value, last_align, conv_w, conv_b, WQ, WV, WU, bias, fc_w,
                bpc=BPC, n_cores=N_CORES):
    """Host-side sharding + layout prep. Returns list of per-core input dicts."""
    t_len = value.shape[1]

    value = np.asarray(value, np.float32)
    la = np.asarray(last_align, np.float32)

    # value^T per batch, bf16
    vT_all = np.ascontiguousarray(value.transpose(0, 2, 1)).astype(BF16)

    nb = vT_all.shape[0]
    la3_all = np.zeros((nb, 3, t_len), np.float32)
    la3_all[:, 0, 1:] = la[:, :-1]
    la3_all[:, 1, :] = la
    la3_all[:, 2, :-1] = la[:, 1:]
    la3_all = la3_all.astype(BF16)

    wvt_h = np.ascontiguousarray(np.asarray(WV, np.float32).T).astype(BF16)
    wqt_h = np.ascontiguousarray(np.asarray(WQ, np.float32).T).astype(BF16)
    wut_h = np.ascontiguousarray(np.asarray(WU, np.float32).T).astype(BF16)
    cw_h = np.ascontiguousarray(np.asarray(conv_w, np.float32)[:, 0, :]).astype(BF16)
    biasT_h = np.ascontiguousarray(
        np.asarray(bias, np.float32).reshape(AC, 128).T
    ).astype(np.float32)
    fcr_h = np.ascontiguousarray(
        np.asarray(fc_w, np.float32)[0].reshape(AC, 128).T
    ).astype(BF16)

    qT_all = np.ascontiguousarray(
        np.asarray(query, np.float32)[:, 0, :].T
    ).astype(BF16)  # [DEC, B]
    cb_h = np.ascontiguousarray(
        np.repeat(np.asarray(conv_b, np.float32)[:, None], bpc, axis=1)
    ).astype(BF16)

    in_maps = []
    for c in range(n_cores):
        bs = slice(c * bpc, (c + 1) * bpc)
        in_maps.append({
            "vT": np.ascontiguousarray(vT_all[bs]),
            "la3": np.ascontiguousarray(la3_all[bs]),
            "wvt": wvt_h,
            "wqt": wqt_h,
            "qT": np.ascontiguousarray(qT_all[:, bs]),
            "wut": wut_h,
            "cw": cw_h,
            "cb": cb_h,
            "biasT": biasT_h,
            "fcr": fcr_h,
        })
    return in_maps


@functools.lru_cache(maxsize=1)
def _get_nc():
    return build_kernel()


def run(inputs, trace=False, **kwargs):
    from concourse.bass_utils import run_bass_kernel_spmd

    nc = _get_nc()
    in_maps = prep_inputs(**inputs)
    res = run_bass_kernel_spmd(
        nc, in_maps, core_ids=list(range(N_CORES)), trace=trace, **kwargs
    )
    ctx = np.concatenate([np.asarray(r["ctx_out"]) for r in res.results], axis=0)
    align = np.concatenate([np.asarray(r["align_out"]) for r in res.results], axis=0)
    return (ctx.astype(np.float32), align.astype(np.float32)), res


def kernel(**inputs):
    (ctx, align), _ = run(inputs, trace=False)
    return ctx, align


# revision 22
# speedup vs baseline: 1.3757x; 1.0469x over previous
"""LocationAwareAttention Trainium2 kernel.

Full-input contract: kernel(**inputs) takes the complete unsharded inputs
(as produced by the problem's setup_inputs) and returns (context, align) as
full-shape fp32 arrays.  Internally the batch dimension (B=32) is sharded
across 8 NeuronCores (4 batches per core); all weights are replicated.

Math (per batch b):
    conv_feat = conv1d(last_align, conv_w, pad=1) + conv_b          [T, K]
    z[t, a]   = (value[t] @ WV^T)[a] + (conv_feat[t] @ WU^T)[a]
                + (query @ WQ^T)[a] + bias[a]
    score[t]  = fc_w . tanh(z[t])
    p         = sigmoid(score);  align = p / sum(p)
    context   = align @ value                                        [ENC]

Device-side layout (per core, per batch):
  - value is staged host-side as value^T (bf16, [ENC, T]) so the big
    projection runs with the contraction dim (ENC) on partitions.
  - conv+WU collapse into a rank-3 term: u[t,:] = sum_j W3[:,j]*la[t+j-1],
    where W3 = WU @ conv_w — computed on device from WU^T and conv_w.
  - z tiles are [a_chunk(128), t(512)] psum banks: 1 K=3 matmul (u term)
    + 8 K=128 matmuls (value^T @ WV^T chunks); per-batch constant
    q + bias + WU@conv_b enters via the tanh activation's per-partition bias.
  - score: 4 fc matmuls (M=1) accumulate into psum [1, 512]; sigmoid (ACT)
    with accum_out produces the per-tile sum of p for free.
  - context: p is bounced through DRAM to broadcast it across all 128
    partitions (with an fp32->bf16 cast in the DMA), then one fused DVE
    scalar_tensor_tensor per e-chunk does the multiply and row-reduce into
    per-subtile partials; a per-batch reduce + 1/sum scaling finishes it.
"""

import os
import sys
import functools

_TRN_REPO = "/opt/trn_rl_repo"
if _TRN_REPO not in sys.path and os.path.isdir(_TRN_REPO):
    sys.path.insert(0, _TRN_REPO)

import numpy as np
import ml_dtypes

BF16 = ml_dtypes.bfloat16

B, T_FULL, DEC, ENC, ATTN, KCONV = 32, 4096, 1024, 1024, 512, 10
N_CORES = 8
BPC = B // N_CORES          # batches per core
TT = 512                    # t-tile (columns per psum bank)
EC = ENC // 128             # e chunks (8)
AC = ATTN // 128            # a chunks (4)


def build_kernel(bpc=BPC, t_len=T_FULL):
    """Build the Bass module for one core handling `bpc` batches of length t_len."""
    import concourse.bass as bass  # noqa: F401
    import concourse.tile as tile
    from concourse import bacc, mybir

    f32 = mybir.dt.float32
    bf16 = mybir.dt.bfloat16
    AF = mybir.ActivationFunctionType
    ALU = mybir.AluOpType

    n_tiles = t_len // TT

    nc = bacc.Bacc(trn_type="TRN2")

    # ---- DRAM I/O ----
    vT = nc.dram_tensor("vT", [bpc, ENC, t_len], bf16, kind="ExternalInput")
    la3 = nc.dram_tensor("la3", [bpc, 3, t_len], bf16, kind="ExternalInput")
    wvt = nc.dram_tensor("wvt", [ENC, ATTN], bf16, kind="ExternalInput")
    wqt = nc.dram_tensor("wqt", [DEC, ATTN], bf16, kind="ExternalInput")
    qT = nc.dram_tensor("qT", [DEC, bpc], bf16, kind="ExternalInput")
    wut = nc.dram_tensor("wut", [KCONV, ATTN], bf16, kind="ExternalInput")
    cw = nc.dram_tensor("cw", [KCONV, 3], bf16, kind="ExternalInput")
    cb = nc.dram_tensor("cb", [KCONV, bpc], bf16, kind="ExternalInput")
    biasT = nc.dram_tensor("biasT", [128, AC], f32, kind="ExternalInput")
    fcr = nc.dram_tensor("fcr", [128, AC], bf16, kind="ExternalInput")

    ctx_out = nc.dram_tensor("ctx_out", [bpc, ENC], f32, kind="ExternalOutput")
    align_out = nc.dram_tensor("align_out", [bpc, t_len], f32, kind="ExternalOutput")

    with tile.TileContext(nc) as tc:
        with (
            tc.tile_pool(name="consts", bufs=1) as consts,
            tc.tile_pool(name="setupp", bufs=1) as setupp,
            tc.tile_pool(name="vtp", bufs=6) as vtp,
            tc.tile_pool(name="la3p", bufs=2) as la3p,
            tc.tile_pool(name="tanhp", bufs=20) as tanhp,
            tc.tile_pool(name="pbp", bufs=3) as pbp,
            tc.tile_pool(name="scrp", bufs=2) as scrp,
            tc.tile_pool(name="batchp", bufs=2) as batchp,
            tc.tile_pool(name="dramp", bufs=2, space="DRAM") as dramp,
            tc.tile_pool(name="psz", bufs=6, space="PSUM") as psz,
            tc.tile_pool(name="pss", bufs=1, space="PSUM") as pss,
            tc.tile_pool(name="psmisc", bufs=1, space="PSUM") as psmisc,
        ):
            # ---- constant loads ----
            wvt_sb = consts.tile([128, EC, ATTN], bf16)
            wvt_r = wvt.rearrange("(c p) a -> p c a", p=128)
            nc.sync.dma_start(wvt_sb[:, :2, :], wvt_r[:, :2, :])
            nc.sync.dma_start(wvt_sb[:, 2:EC, :], wvt_r[:, 2:EC, :])
            wqt_sb = setupp.tile([128, DEC // 128, ATTN], bf16)
            nc.scalar.dma_start(wqt_sb, wqt.rearrange("(c p) a -> p c a", p=128))
            qT_sb = setupp.tile([128, DEC // 128, bpc], bf16)
            nc.scalar.dma_start(qT_sb, qT.rearrange("(c p) b -> p c b", p=128))
            wut_sb = consts.tile([KCONV, ATTN], bf16)
            nc.scalar.dma_start(wut_sb, wut[:])
            cw_sb = consts.tile([KCONV, 3], bf16)
            nc.scalar.dma_start(cw_sb, cw[:])
            cb_sb = consts.tile([KCONV, bpc], bf16)
            nc.scalar.dma_start(cb_sb, cb[:])
            biasT_sb = consts.tile([128, AC], f32)
            nc.scalar.dma_start(biasT_sb, biasT[:])
            fcr_sb = consts.tile([128, AC], bf16)
            nc.scalar.dma_start(fcr_sb, fcr[:])

            ones_sb = consts.tile([1, 128], f32)
            nc.vector.memset(ones_sb, 1.0)
            ident_sb = consts.tile([128, 128], f32)
            from concourse.masks import make_identity
            make_identity(nc, ident_sb)

            # ---- W3T = cw^T @ WU^T : [3, ATTN] ----
            w3_ps = psmisc.tile([3, ATTN], f32, tag="mps")
            nc.tensor.matmul(w3_ps, lhsT=cw_sb, rhs=wut_sb, start=True, stop=True)
            w3_sb = consts.tile([3, ATTN], bf16)
            nc.scalar.copy(w3_sb, w3_ps)

            # ---- cT[a, chunk, b] = (query @ WQ^T + WU @ conv_b + bias)^T ----
            cT_sb = consts.tile([128, AC, bpc], f32)
            for a in range(AC):
                qt_ps = psmisc.tile([128, bpc], f32, tag="mps")
                for c in range(DEC // 128):
                    nc.tensor.matmul(
                        qt_ps,
                        lhsT=wqt_sb[:, c, a * 128:(a + 1) * 128],
                        rhs=qT_sb[:, c, :],
                        start=(c == 0),
                        stop=False,
                    )
                nc.tensor.matmul(
                    qt_ps,
                    lhsT=wut_sb[:, a * 128:(a + 1) * 128],
                    rhs=cb_sb,
                    start=False,
                    stop=True,
                )
                nc.scalar.activation(
                    cT_sb[:, a, :], qt_ps, AF.Identity,
                    bias=biasT_sb[:, a:a + 1], scale=1.0,
                )

            # ---- main loops ----
            HT = min(1024, t_len)       # columns per vT DMA tile
            spt = HT // TT              # subtiles per DMA tile
            n_ht = t_len // HT
            for b in range(bpc):
                la3_sb = la3p.tile([3, t_len], bf16)
                nc.sync.dma_start(la3_sb, la3[b])

                p_row = batchp.tile([1, t_len], f32, tag="p_row")
                p_dram = dramp.tile([t_len], f32, tag="p_dram")
                psums = batchp.tile([1, n_tiles], f32, tag="psums")
                parts = batchp.tile([128, EC, n_tiles], f32, tag="parts")

                ht_tiles = {}
                vT_r = vT[b].rearrange("(c p) t -> p c t", p=128)
                for h in range(n_ht):
                    vt_tile = vtp.tile([128, EC, HT], bf16, tag="vt",
                                       name=f"vt_{b}_{h}")
                    if b == 0 and h == 0:
                        half = HT // 2
                        nc.sync.dma_start(vt_tile[:, :, :half],
                                          vT_r[:, :, :half])
                        nc.sync.dma_start(vt_tile[:, :, half:HT],
                                          vT_r[:, :, half:HT])
                    else:
                        nc.sync.dma_start(
                            vt_tile, vT_r[:, :, h * HT:(h + 1) * HT]
                        )
                    ht_tiles[h] = vt_tile

                def vt_sl(ti):
                    """(tile, free-slice) of subtile ti's columns."""
                    h, off = divmod(ti * TT, HT)
                    return ht_tiles[h], slice(off, off + TT)

                # group plan: JJ subtiles share each stationary load; taper
                # the final groups of the last batch to shorten the drain.
                plan = []
                rem = n_tiles
                while rem > 0:
                    jj = min(4, rem)
                    plan.append(jj)
                    rem -= jj
                if b == bpc - 1 and plan and plan[-1] == 4:
                    plan = plan[:-1] + [2, 1, 1]

                s0 = 0
                for gi, jj in enumerate(plan):
                    subt = [s0 + j for j in range(jj)]
                    s0 += jj

                    tanhs = {}
                    for a in range(AC):
                        zs = []
                        for j, ti in enumerate(subt):
                            z_ps = psz.tile([128, TT], f32, tag="z",
                                            name=f"z_{b}_{gi}_{a}_{j}")
                            zs.append(z_ps)
                            nc.tensor.matmul(
                                z_ps,
                                lhsT=w3_sb[:, a * 128:(a + 1) * 128],
                                rhs=la3_sb[:, ti * TT:(ti + 1) * TT],
                                start=True,
                                stop=False,
                            )
                        for c in range(EC):
                            for j, ti in enumerate(subt):
                                vt_t, fsl = vt_sl(ti)
                                nc.tensor.matmul(
                                    zs[j],
                                    lhsT=wvt_sb[:, c, a * 128:(a + 1) * 128],
                                    rhs=vt_t[:, c, fsl],
                                    start=False,
                                    stop=(c == EC - 1),
                                )
                        for j, ti in enumerate(subt):
                            tanh_sb = tanhp.tile([128, TT], bf16, tag="tanh",
                                                 name=f"tanh_{b}_{gi}_{a}_{j}")
                            nc.scalar.activation(
                                tanh_sb, zs[j], AF.Tanh,
                                bias=cT_sb[:, a, b:b + 1], scale=1.0,
                            )
                            tanhs[(a, j)] = tanh_sb

                    for j, ti in enumerate(subt):
                        tsl = slice(ti * TT, (ti + 1) * TT)
                        s_ps = pss.tile([1, TT], f32, tag="s",
                                        name=f"s_{b}_{gi}_{j}")
                        for a in range(AC):
                            nc.tensor.matmul(
                                s_ps,
                                lhsT=fcr_sb[:, a:a + 1],
                                rhs=tanhs[(a, j)],
                                start=(a == 0),
                                stop=(a == AC - 1),
                            )

                        nc.scalar.activation(
                            p_row[0:1, tsl], s_ps, AF.Sigmoid,
                            accum_out=psums[0:1, ti:ti + 1],
                        )

                        pb_sb = pbp.tile([128, TT], bf16, tag="pb_sb",
                                         name=f"pb_{b}_{gi}_{j}")
                        nc.scalar.dma_start(
                            p_dram[tsl][None, :], p_row[0:1, tsl]
                        )
                        nc.gpsimd.dma_start(
                            pb_sb,
                            p_dram[tsl][None, :].to_broadcast([128, TT]),
                        )

                        scr = scrp.tile([128, TT], bf16, tag="scr",
                                        name=f"scr_{b}_{gi}_{j}")
                        vt_t, fsl = vt_sl(ti)
                        for c in range(EC):
                            nc.vector.scalar_tensor_tensor(
                                out=scr,
                                in0=vt_t[:, c, fsl],
                                scalar=1.0,
                                in1=pb_sb,
                                op0=ALU.mult,
                                op1=ALU.mult,
                                accum_out=parts[:, c, ti:ti + 1],
                            )

                # ---- batch epilogue ----
                sum1 = batchp.tile([1, 1], f32, tag="sum1")
                nc.vector.reduce_sum(sum1, psums, axis=mybir.AxisListType.X)
                inv1 = batchp.tile([1, 1], f32, tag="inv1")
                nc.vector.reciprocal(inv1, sum1)

                nc.vector.tensor_scalar_mul(p_row, p_row, inv1)
                if b == bpc - 1:
                    nc.scalar.dma_start(align_out[b:b + 1, :], p_row)
                else:
                    nc.gpsimd.dma_start(align_out[b:b + 1, :], p_row)

                invp_sb = batchp.tile([128, 1], f32, tag="invp_sb")
                if b == bpc - 1:
                    invp_ps = psmisc.tile([128, 1], f32, tag="mps",
                                          name=f"invp_{b}")
                    nc.tensor.matmul(invp_ps, lhsT=ones_sb, rhs=inv1,
                                     start=True, stop=True)
                    nc.scalar.copy(invp_sb, invp_ps)
                else:
                    i_dram = dramp.tile([1], f32, tag="i_dram")
                    nc.gpsimd.dma_start(i_dram[None, :], inv1)
                    nc.gpsimd.dma_start(
                        invp_sb, i_dram[None, :].to_broadcast([128, 1])
                    )

                ctx_acc = batchp.tile([128, EC], f32, tag="ctx_acc")
                for c in range(EC):
                    nc.vector.reduce_sum(
                        ctx_acc[:, c:c + 1], parts[:, c, :],
                        axis=mybir.AxisListType.X,
                    )
                ctx_sb = batchp.tile([128, EC], f32, tag="ctx_sb")
                nc.vector.tensor_scalar_mul(ctx_sb, ctx_acc, invp_sb)
                if b == bpc - 1:
                    ctxT_ps = psmisc.tile([EC, 128], f32, tag="mps",
                                          name=f"ctxT_{b}")
                    nc.tensor.transpose(ctxT_ps, ctx_sb, ident_sb)
                    ctxT_sb = batchp.tile([EC, 128], f32, tag="ctxT_sb")
                    nc.scalar.copy(ctxT_sb, ctxT_ps)
                    nc.scalar.dma_start(
                        ctx_out[b].rearrange("(c p) -> c p", p=128), ctxT_sb
                    )
                else:
                    nc.gpsimd.dma_start(
                        ctx_out[b].rearrange("(c p) -> p c", p=128), ctx_sb
                    )

    nc.compile()
    return nc


def prep_inputs(query, value, last_align, conv_w, conv_b, WQ, WV, WU, bias, fc_w,
                bpc=BPC, n_cores=N_CORES):
    """Host-side sharding + layout prep. Returns list of per-core input dicts."""
    t_len = value.shape[1]

    value = np.asarray(value, np.float32)
    la = np.asarray(last_align, np.float32)

    # value^T per batch, bf16
    vT_all = np.ascontiguousarray(value.transpose(0, 2, 1)).astype(BF16)

    nb = vT_all.shape[0]
    la3_all = np.zeros((nb, 3, t_len), np.float32)
    la3_all[:, 0, 1:] = la[:, :-1]
    la3_all[:, 1, :] = la
    la3_all[:, 2, :-1] = la[:, 1:]
    la3_all = la3_all.astype(BF16)

    wvt_h = np.ascontiguousarray(np.asarray(WV, np.float32).T).astype(BF16)
    wqt_h = np.ascontiguousarray(np.asarray(WQ, np.float32).T).astype(BF16)
    wut_h = np.ascontiguousarray(np.asarray(WU, np.float32).T).astype(BF16)
    cw_h = np.ascontiguousarray(np.asarray(conv_w, np.float32)[:, 0, :]).astype(BF16)
    biasT_h = np.ascontiguousarray(
        np.asarray(bias, np.float32).reshape(AC, 128).T
    ).astype(np.float32)
    fcr_h = np.ascontiguousarray(
        np.asarray(fc_w, np.float32)[0].reshape(AC, 128).T
    ).astype(BF16)

    qT_all = np.ascontiguousarray(
        np.asarray(query, np.float32)[:, 0, :].T
    ).astype(BF16)  # [DEC, B]
    cb_h = np.ascontiguousarray(
        np.repeat(np.asarray(conv_b, np.float32)[:, None], bpc, axis=1)
    ).astype(BF16)

    in_maps = []
    for c in range(n_cores):
        bs = slice(c * bpc, (c + 1) * bpc)
        in_maps.append({
            "vT": np.ascontiguousarray(vT_all[bs]),
            "la3": np.ascontiguousarray(la3_all[bs]),
            "wvt": wvt_h,
            "wqt": wqt_h,
            "qT": np.ascontiguousarray(qT_all[:, bs]),
            "wut": wut_h,
            "cw": cw_h,
            "cb": cb_h,
            "biasT": biasT_h,
            "fcr": fcr_h,
        })
    return in_maps


@functools.lru_cache(maxsize=1)
def _get_nc():
    return build_kernel()


def run(inputs, trace=False, **kwargs):
    from concourse.bass_utils import run_bass_kernel_spmd

    nc = _get_nc()
    in_maps = prep_inputs(**inputs)
    res = run_bass_kernel_spmd(
        nc, in_maps, core_ids=list(range(N_CORES)), trace=trace, **kwargs
    )
    ctx = np.concatenate([np.asarray(r["ctx_out"]) for r in res.results], axis=0)
    align = np.concatenate([np.asarray(r["align_out"]) for r in res.results], axis=0)
    return (ctx.astype(np.float32), align.astype(np.float32)), res


def kernel(**inputs):
    (ctx, align), _ = run(inputs, trace=False)
    return ctx, align


# revision 23
# speedup vs baseline: 1.3806x; 1.0036x over previous
"""LocationAwareAttention Trainium2 kernel.

Full-input contract: kernel(**inputs) takes the complete unsharded inputs
(as produced by the problem's setup_inputs) and returns (context, align) as
full-shape fp32 arrays.  Internally the batch dimension (B=32) is sharded
across 8 NeuronCores (4 batches per core); all weights are replicated.

Math (per batch b):
    conv_feat = conv1d(last_align, conv_w, pad=1) + conv_b          [T, K]
    z[t, a]   = (value[t] @ WV^T)[a] + (conv_feat[t] @ WU^T)[a]
                + (query @ WQ^T)[a] + bias[a]
    score[t]  = fc_w . tanh(z[t])
    p         = sigmoid(score);  align = p / sum(p)
    context   = align @ value                                        [ENC]

Device-side layout (per core, per batch):
  - value is staged host-side as value^T (bf16, [ENC, T]) so the big
    projection runs with the contraction dim (ENC) on partitions.
  - conv+WU collapse into a rank-3 term: u[t,:] = sum_j W3[:,j]*la[t+j-1],
    where W3 = WU @ conv_w — computed on device from WU^T and conv_w.
  - z tiles are [a_chunk(128), t(512)] psum banks: 1 K=3 matmul (u term)
    + 8 K=128 matmuls (value^T @ WV^T chunks); per-batch constant
    q + bias + WU@conv_b enters via the tanh activation's per-partition bias.
  - score: 4 fc matmuls (M=1) accumulate into psum [1, 512]; sigmoid (ACT)
    with accum_out produces the per-tile sum of p for free.
  - context: p is bounced through DRAM to broadcast it across all 128
    partitions (with an fp32->bf16 cast in the DMA), then one fused DVE
    scalar_tensor_tensor per e-chunk does the multiply and row-reduce into
    per-subtile partials; a per-batch reduce + 1/sum scaling finishes it.
"""

import os
import sys
import functools

_TRN_REPO = "/opt/trn_rl_repo"
if _TRN_REPO not in sys.path and os.path.isdir(_TRN_REPO):
    sys.path.insert(0, _TRN_REPO)

import numpy as np
import ml_dtypes

BF16 = ml_dtypes.bfloat16

B, T_FULL, DEC, ENC, ATTN, KCONV = 32, 4096, 1024, 1024, 512, 10
N_CORES = 8
BPC = B // N_CORES          # batches per core
TT = 512                    # t-tile (columns per psum bank)
EC = ENC // 128             # e chunks (8)
AC = ATTN // 128            # a chunks (4)


def build_kernel(bpc=BPC, t_len=T_FULL):
    """Build the Bass module for one core handling `bpc` batches of length t_len."""
    import concourse.bass as bass  # noqa: F401
    import concourse.tile as tile
    from concourse import bacc, mybir

    f32 = mybir.dt.float32
    bf16 = mybir.dt.bfloat16
    AF = mybir.ActivationFunctionType
    ALU = mybir.AluOpType

    n_tiles = t_len // TT

    nc = bacc.Bacc(trn_type="TRN2")

    # ---- DRAM I/O ----
    vT = nc.dram_tensor("vT", [bpc, ENC, t_len], bf16, kind="ExternalInput")
    la3 = nc.dram_tensor("la3", [bpc, 3, t_len], bf16, kind="ExternalInput")
    wvt = nc.dram_tensor("wvt", [ENC, ATTN], bf16, kind="ExternalInput")
    wqt = nc.dram_tensor("wqt", [DEC, ATTN], bf16, kind="ExternalInput")
    qT = nc.dram_tensor("qT", [DEC, bpc], bf16, kind="ExternalInput")
    wut = nc.dram_tensor("wut", [KCONV, ATTN], bf16, kind="ExternalInput")
    cw = nc.dram_tensor("cw", [KCONV, 3], bf16, kind="ExternalInput")
    cb = nc.dram_tensor("cb", [KCONV, bpc], bf16, kind="ExternalInput")
    biasT = nc.dram_tensor("biasT", [128, AC], f32, kind="ExternalInput")
    fcr = nc.dram_tensor("fcr", [128, AC], bf16, kind="ExternalInput")

    ctx_out = nc.dram_tensor("ctx_out", [bpc, ENC], f32, kind="ExternalOutput")
    align_out = nc.dram_tensor("align_out", [bpc, t_len], f32, kind="ExternalOutput")

    with tile.TileContext(nc) as tc:
        with (
            tc.tile_pool(name="consts", bufs=1) as consts,
            tc.tile_pool(name="setupp", bufs=1) as setupp,
            tc.tile_pool(name="vtp", bufs=6) as vtp,
            tc.tile_pool(name="la3p", bufs=2) as la3p,
            tc.tile_pool(name="tanhp", bufs=20) as tanhp,
            tc.tile_pool(name="pbp", bufs=3) as pbp,
            tc.tile_pool(name="scrp", bufs=2) as scrp,
            tc.tile_pool(name="batchp", bufs=2) as batchp,
            tc.tile_pool(name="dramp", bufs=2, space="DRAM") as dramp,
            tc.tile_pool(name="psz", bufs=6, space="PSUM") as psz,
            tc.tile_pool(name="pss", bufs=1, space="PSUM") as pss,
            tc.tile_pool(name="psmisc", bufs=1, space="PSUM") as psmisc,
        ):
            # ---- constant loads ----
            wvt_sb = consts.tile([128, EC, ATTN], bf16)
            wvt_r = wvt.rearrange("(c p) a -> p c a", p=128)
            nc.sync.dma_start(wvt_sb[:, :2, :], wvt_r[:, :2, :])
            nc.sync.dma_start(wvt_sb[:, 2:EC, :], wvt_r[:, 2:EC, :])
            wqt_sb = setupp.tile([128, DEC // 128, ATTN], bf16)
            nc.scalar.dma_start(wqt_sb, wqt.rearrange("(c p) a -> p c a", p=128))
            qT_sb = setupp.tile([128, DEC // 128, bpc], bf16)
            nc.scalar.dma_start(qT_sb, qT.rearrange("(c p) b -> p c b", p=128))
            wut_sb = consts.tile([KCONV, ATTN], bf16)
            nc.scalar.dma_start(wut_sb, wut[:])
            cw_sb = consts.tile([KCONV, 3], bf16)
            nc.scalar.dma_start(cw_sb, cw[:])
            cb_sb = consts.tile([KCONV, bpc], bf16)
            nc.scalar.dma_start(cb_sb, cb[:])
            biasT_sb = consts.tile([128, AC], f32)
            nc.scalar.dma_start(biasT_sb, biasT[:])
            fcr_sb = consts.tile([128, AC], bf16)
            nc.scalar.dma_start(fcr_sb, fcr[:])

            ones_sb = consts.tile([1, 128], f32)
            nc.vector.memset(ones_sb, 1.0)
            ident_sb = consts.tile([128, 128], f32)
            from concourse.masks import make_identity
            make_identity(nc, ident_sb)

            # ---- W3T = cw^T @ WU^T : [3, ATTN] ----
            w3_ps = psmisc.tile([3, ATTN], f32, tag="mps")
            nc.tensor.matmul(w3_ps, lhsT=cw_sb, rhs=wut_sb, start=True, stop=True)
            w3_sb = consts.tile([3, ATTN], bf16)
            nc.scalar.copy(w3_sb, w3_ps)

            # ---- cT[a, chunk, b] = (query @ WQ^T + WU @ conv_b + bias)^T ----
            cT_sb = consts.tile([128, AC, bpc], f32)
            for a in range(AC):
                qt_ps = psmisc.tile([128, bpc], f32, tag="mps")
                for c in range(DEC // 128):
                    nc.tensor.matmul(
                        qt_ps,
                        lhsT=wqt_sb[:, c, a * 128:(a + 1) * 128],
                        rhs=qT_sb[:, c, :],
                        start=(c == 0),
                        stop=False,
                    )
                nc.tensor.matmul(
                    qt_ps,
                    lhsT=wut_sb[:, a * 128:(a + 1) * 128],
                    rhs=cb_sb,
                    start=False,
                    stop=True,
                )
                nc.scalar.activation(
                    cT_sb[:, a, :], qt_ps, AF.Identity,
                    bias=biasT_sb[:, a:a + 1], scale=1.0,
                )

            # ---- main loops ----
            HT = min(1024, t_len)       # columns per vT DMA tile
            spt = HT // TT              # subtiles per DMA tile
            n_ht = t_len // HT
            for b in range(bpc):
                la3_sb = la3p.tile([3, t_len], bf16)
                nc.sync.dma_start(la3_sb, la3[b])

                p_row = batchp.tile([1, t_len], f32, tag="p_row")
                p_dram = dramp.tile([t_len], f32, tag="p_dram")
                p_bdram = dramp.tile([t_len], bf16, tag="p_bdram")
                psums = batchp.tile([1, n_tiles], f32, tag="psums")
                parts = batchp.tile([128, EC, n_tiles], f32, tag="parts")

                ht_tiles = {}
                vT_r = vT[b].rearrange("(c p) t -> p c t", p=128)
                for h in range(n_ht):
                    vt_tile = vtp.tile([128, EC, HT], bf16, tag="vt",
                                       name=f"vt_{b}_{h}")
                    if b == 0 and h == 0:
                        half = HT // 2
                        nc.sync.dma_start(vt_tile[:, :, :half],
                                          vT_r[:, :, :half])
                        nc.sync.dma_start(vt_tile[:, :, half:HT],
                                          vT_r[:, :, half:HT])
                    else:
                        nc.sync.dma_start(
                            vt_tile, vT_r[:, :, h * HT:(h + 1) * HT]
                        )
                    ht_tiles[h] = vt_tile

                def vt_sl(ti):
                    """(tile, free-slice) of subtile ti's columns."""
                    h, off = divmod(ti * TT, HT)
                    return ht_tiles[h], slice(off, off + TT)

                # group plan: JJ subtiles share each stationary load; taper
                # the final groups of the last batch to shorten the drain.
                plan = []
                rem = n_tiles
                while rem > 0:
                    jj = min(4, rem)
                    plan.append(jj)
                    rem -= jj
                if b == bpc - 1 and plan and plan[-1] == 4:
                    plan = plan[:-1] + [2, 1, 1]

                s0 = 0
                for gi, jj in enumerate(plan):
                    subt = [s0 + j for j in range(jj)]
                    s0 += jj

                    tanhs = {}
                    for a in range(AC):
                        zs = []
                        for j, ti in enumerate(subt):
                            z_ps = psz.tile([128, TT], f32, tag="z",
                                            name=f"z_{b}_{gi}_{a}_{j}")
                            zs.append(z_ps)
                            nc.tensor.matmul(
                                z_ps,
                                lhsT=w3_sb[:, a * 128:(a + 1) * 128],
                                rhs=la3_sb[:, ti * TT:(ti + 1) * TT],
                                start=True,
                                stop=False,
                            )
                        for c in range(EC):
                            for j, ti in enumerate(subt):
                                vt_t, fsl = vt_sl(ti)
                                nc.tensor.matmul(
                                    zs[j],
                                    lhsT=wvt_sb[:, c, a * 128:(a + 1) * 128],
                                    rhs=vt_t[:, c, fsl],
                                    start=False,
                                    stop=(c == EC - 1),
                                )
                        for j, ti in enumerate(subt):
                            tanh_sb = tanhp.tile([128, TT], bf16, tag="tanh",
                                                 name=f"tanh_{b}_{gi}_{a}_{j}")
                            nc.scalar.activation(
                                tanh_sb, zs[j], AF.Tanh,
                                bias=cT_sb[:, a, b:b + 1], scale=1.0,
                            )
                            tanhs[(a, j)] = tanh_sb

                    for j, ti in enumerate(subt):
                        tsl = slice(ti * TT, (ti + 1) * TT)
                        s_ps = pss.tile([1, TT], f32, tag="s",
                                        name=f"s_{b}_{gi}_{j}")
                        for a in range(AC):
                            nc.tensor.matmul(
                                s_ps,
                                lhsT=fcr_sb[:, a:a + 1],
                                rhs=tanhs[(a, j)],
                                start=(a == 0),
                                stop=(a == AC - 1),
                            )

                        nc.scalar.activation(
                            p_row[0:1, tsl], s_ps, AF.Sigmoid,
                            accum_out=psums[0:1, ti:ti + 1],
                        )

                        pb_sb = pbp.tile([128, TT], bf16, tag="pb_sb",
                                         name=f"pb_{b}_{gi}_{j}")
                        if b == bpc - 1:
                            p_bf = pbp.tile([1, TT], bf16, tag="p_bf",
                                            name=f"pbf_{b}_{gi}_{j}")
                            nc.scalar.copy(p_bf, p_row[0:1, tsl])
                            nc.scalar.dma_start(
                                p_bdram[tsl][None, :], p_bf
                            )
                            nc.sync.dma_start(
                                pb_sb,
                                p_bdram[tsl][None, :].to_broadcast([128, TT]),
                            )
                        else:
                            nc.scalar.dma_start(
                                p_dram[tsl][None, :], p_row[0:1, tsl]
                            )
                            nc.gpsimd.dma_start(
                                pb_sb,
                                p_dram[tsl][None, :].to_broadcast([128, TT]),
                            )

                        scr = scrp.tile([128, TT], bf16, tag="scr",
                                        name=f"scr_{b}_{gi}_{j}")
                        vt_t, fsl = vt_sl(ti)
                        for c in range(EC):
                            nc.vector.scalar_tensor_tensor(
                                out=scr,
                                in0=vt_t[:, c, fsl],
                                scalar=1.0,
                                in1=pb_sb,
                                op0=ALU.mult,
                                op1=ALU.mult,
                                accum_out=parts[:, c, ti:ti + 1],
                            )

                # ---- batch epilogue ----
                sum1 = batchp.tile([1, 1], f32, tag="sum1")
                nc.vector.reduce_sum(sum1, psums, axis=mybir.AxisListType.X)
                inv1 = batchp.tile([1, 1], f32, tag="inv1")
                nc.vector.reciprocal(inv1, sum1)

                nc.vector.tensor_scalar_mul(p_row, p_row, inv1)
                if b == bpc - 1:
                    nc.scalar.dma_start(align_out[b:b + 1, :], p_row)
                else:
                    nc.gpsimd.dma_start(align_out[b:b + 1, :], p_row)

                invp_sb = batchp.tile([128, 1], f32, tag="invp_sb")
                if b == bpc - 1:
                    invp_ps = psmisc.tile([128, 1], f32, tag="mps",
                                          name=f"invp_{b}")
                    nc.tensor.matmul(invp_ps, lhsT=ones_sb, rhs=inv1,
                                     start=True, stop=True)
                    nc.scalar.copy(invp_sb, invp_ps)
                else:
                    i_dram = dramp.tile([1], f32, tag="i_dram")
                    nc.gpsimd.dma_start(i_dram[None, :], inv1)
                    nc.gpsimd.dma_start(
                        invp_sb, i_dram[None, :].to_broadcast([128, 1])
                    )

                ctx_acc = batchp.tile([128, EC], f32, tag="ctx_acc")
                for c in range(EC):
                    nc.vector.reduce_sum(
                        ctx_acc[:, c:c + 1], parts[:, c, :],
                        axis=mybir.AxisListType.X,
                    )
                ctx_sb = batchp.tile([128, EC], f32, tag="ctx_sb")
                nc.vector.tensor_scalar_mul(ctx_sb, ctx_acc, invp_sb)
                if b == bpc - 1:
                    ctxT_ps = psmisc.tile([EC, 128], f32, tag="mps",
                                          name=f"ctxT_{b}")
                    nc.tensor.transpose(ctxT_ps, ctx_sb, ident_sb)
                    ctxT_sb = batchp.tile([EC, 128], f32, tag="ctxT_sb")
                    nc.scalar.copy(ctxT_sb, ctxT_ps)
                    nc.scalar.dma_start(
                        ctx_out[b].rearrange("(c p) -> c p", p=128), ctxT_sb
                    )
                else:
                    nc.gpsimd.dma_start(
                        ctx_out[b].rearrange("(c p) -> p c", p=128), ctx_sb
                    )

    nc.compile()
    return nc


def prep_inputs(query, value, last_align, conv_w, conv_b, WQ, WV, WU, bias, fc_w,
                bpc=BPC, n_cores=N_CORES):
    """Host-side sharding + layout prep. Returns list of per-core input dicts."""
    t_len = value.shape[1]

    value = np.asarray(value, np.float32)
    la = np.asarray(last_align, np.float32)

    # value^T per batch, bf16
    vT_all = np.ascontiguousarray(value.transpose(0, 2, 1)).astype(BF16)

    nb = vT_all.shape[0]
    la3_all = np.zeros((nb, 3, t_len), np.float32)
    la3_all[:, 0, 1:] = la[:, :-1]
    la3_all[:, 1, :] = la
    la3_all[:, 2, :-1] = la[:, 1:]
    la3_all = la3_all.astype(BF16)

    wvt_h = np.ascontiguousarray(np.asarray(WV, np.float32).T).astype(BF16)
    wqt_h = np.ascontiguousarray(np.asarray(WQ, np.float32).T).astype(BF16)
    wut_h = np.ascontiguousarray(np.asarray(WU, np.float32).T).astype(BF16)
    cw_h = np.ascontiguousarray(np.asarray(conv_w, np.float32)[:, 0, :]).astype(BF16)
    biasT_h = np.ascontiguousarray(
        np.asarray(bias, np.float32).reshape(AC, 128).T
    ).astype(np.float32)
    fcr_h = np.ascontiguousarray(
        np.asarray(fc_w, np.float32)[0].reshape(AC, 128).T
    ).astype(BF16)

    qT_all = np.ascontiguousarray(
        np.asarray(query, np.float32)[:, 0, :].T
    ).astype(BF16)  # [DEC, B]
    cb_h = np.ascontiguousarray(
        np.repeat(np.asarray(conv_b, np.float32)[:, None], bpc, axis=1)
    ).astype(BF16)

    in_maps = []
    for c in range(n_cores):
        bs = slice(c * bpc, (c + 1) * bpc)
        in_maps.append({
            "vT": np.ascontiguousarray(vT_all[bs]),
            "la3": np.ascontiguousarray(la3_all[bs]),
            "wvt": wvt_h,
            "wqt": wqt_h,
            "qT": np.ascontiguousarray(qT_all[:, bs]),
            "wut": wut_h,
            "cw": cw_h,
            "cb": cb_h,
            "biasT": biasT_h,
            "fcr": fcr_h,
        })
    return in_maps


@functools.lru_cache(maxsize=1)
def _get_nc():
    return build_kernel()


def run(inputs, trace=False, **kwargs):
    from concourse.bass_utils import run_bass_kernel_spmd

    nc = _get_nc()
    in_maps = prep_inputs(**inputs)
    res = run_bass_kernel_spmd(
        nc, in_maps, core_ids=list(range(N_CORES)), trace=trace, **kwargs
    )
    ctx = np.concatenate([np.asarray(r["ctx_out"]) for r in res.results], axis=0)
    align = np.concatenate([np.asarray(r["align_out"]) for r in res.results], axis=0)
    return (ctx.astype(np.float32), align.astype(np.float32)), res


def kernel(**inputs):
    (ctx, align), _ = run(inputs, trace=False)
    return ctx, align


# revision 24
# speedup vs baseline: 1.4007x; 1.0146x over previous
"""LocationAwareAttention Trainium2 kernel.

Full-input contract: kernel(**inputs) takes the complete unsharded inputs
(as produced by the problem's setup_inputs) and returns (context, align) as
full-shape fp32 arrays.  Internally the batch dimension (B=32) is sharded
across 8 NeuronCores (4 batches per core); all weights are replicated.

Math (per batch b):
    conv_feat = conv1d(last_align, conv_w, pad=1) + conv_b          [T, K]
    z[t, a]   = (value[t] @ WV^T)[a] + (conv_feat[t] @ WU^T)[a]
                + (query @ WQ^T)[a] + bias[a]
    score[t]  = fc_w . tanh(z[t])
    p         = sigmoid(score);  align = p / sum(p)
    context   = align @ value                                        [ENC]

Device-side layout (per core, per batch):
  - value is staged host-side as value^T (bf16, [ENC, T]) so the big
    projection runs with the contraction dim (ENC) on partitions.
  - conv+WU collapse into a rank-3 term: u[t,:] = sum_j W3[:,j]*la[t+j-1],
    where W3 = WU @ conv_w — computed on device from WU^T and conv_w.
  - z tiles are [a_chunk(128), t(512)] psum banks: 1 K=3 matmul (u term)
    + 8 K=128 matmuls (value^T @ WV^T chunks); per-batch constant
    q + bias + WU@conv_b enters via the tanh activation's per-partition bias.
  - score: 4 fc matmuls (M=1) accumulate into psum [1, 512]; sigmoid (ACT)
    with accum_out produces the per-tile sum of p for free.
  - context: p is bounced through DRAM to broadcast it across all 128
    partitions (with an fp32->bf16 cast in the DMA), then one fused DVE
    scalar_tensor_tensor per e-chunk does the multiply and row-reduce into
    per-subtile partials; a per-batch reduce + 1/sum scaling finishes it.
"""

import os
import sys
import functools

_TRN_REPO = "/opt/trn_rl_repo"
if _TRN_REPO not in sys.path and os.path.isdir(_TRN_REPO):
    sys.path.insert(0, _TRN_REPO)

import numpy as np
import ml_dtypes

BF16 = ml_dtypes.bfloat16

B, T_FULL, DEC, ENC, ATTN, KCONV = 32, 4096, 1024, 1024, 512, 10
N_CORES = 8
BPC = B // N_CORES          # batches per core
TT = 512                    # t-tile (columns per psum bank)
EC = ENC // 128             # e chunks (8)
AC = ATTN // 128            # a chunks (4)


def build_kernel(bpc=BPC, t_len=T_FULL):
    """Build the Bass module for one core handling `bpc` batches of length t_len."""
    import concourse.bass as bass  # noqa: F401
    import concourse.tile as tile
    from concourse import bacc, mybir

    f32 = mybir.dt.float32
    bf16 = mybir.dt.bfloat16
    AF = mybir.ActivationFunctionType
    ALU = mybir.AluOpType

    n_tiles = t_len // TT

    nc = bacc.Bacc(trn_type="TRN2")

    # ---- DRAM I/O ----
    vT = nc.dram_tensor("vT", [bpc, ENC, t_len], bf16, kind="ExternalInput")
    la3 = nc.dram_tensor("la3", [bpc, 3, t_len], bf16, kind="ExternalInput")
    wvt = nc.dram_tensor("wvt", [ENC, ATTN], bf16, kind="ExternalInput")
    wqt = nc.dram_tensor("wqt", [DEC, ATTN], bf16, kind="ExternalInput")
    qT = nc.dram_tensor("qT", [DEC, bpc], bf16, kind="ExternalInput")
    wut = nc.dram_tensor("wut", [KCONV, ATTN], bf16, kind="ExternalInput")
    cw = nc.dram_tensor("cw", [KCONV, 3], bf16, kind="ExternalInput")
    cb = nc.dram_tensor("cb", [KCONV, bpc], bf16, kind="ExternalInput")
    biasT = nc.dram_tensor("biasT", [128, AC], f32, kind="ExternalInput")
    fcr = nc.dram_tensor("fcr", [128, AC], bf16, kind="ExternalInput")

    ctx_out = nc.dram_tensor("ctx_out", [bpc, ENC], f32, kind="ExternalOutput")
    align_out = nc.dram_tensor("align_out", [bpc, t_len], f32, kind="ExternalOutput")

    with tile.TileContext(nc) as tc:
        with (
            tc.tile_pool(name="consts", bufs=1) as consts,
            tc.tile_pool(name="setupp", bufs=1) as setupp,
            tc.tile_pool(name="vtp", bufs=6) as vtp,
            tc.tile_pool(name="la3p", bufs=2) as la3p,
            tc.tile_pool(name="tanhp", bufs=20) as tanhp,
            tc.tile_pool(name="pbp", bufs=3) as pbp,
            tc.tile_pool(name="scrp", bufs=2) as scrp,
            tc.tile_pool(name="batchp", bufs=2) as batchp,
            tc.tile_pool(name="dramp", bufs=2, space="DRAM") as dramp,
            tc.tile_pool(name="psz", bufs=6, space="PSUM") as psz,
            tc.tile_pool(name="pss", bufs=1, space="PSUM") as pss,
            tc.tile_pool(name="psmisc", bufs=1, space="PSUM") as psmisc,
        ):
            # ---- constant loads ----
            wvt_sb = consts.tile([128, EC, ATTN], bf16)
            wvt_r = wvt.rearrange("(c p) a -> p c a", p=128)
            nc.sync.dma_start(wvt_sb[:, :2, :], wvt_r[:, :2, :])
            nc.sync.dma_start(wvt_sb[:, 2:EC, :], wvt_r[:, 2:EC, :])
            wut_sb = consts.tile([KCONV, ATTN], bf16)
            nc.scalar.dma_start(wut_sb, wut[:])
            cw_sb = consts.tile([KCONV, 3], bf16)
            nc.scalar.dma_start(cw_sb, cw[:])
            wqt_sb = setupp.tile([128, DEC // 128, ATTN], bf16)
            nc.scalar.dma_start(wqt_sb, wqt.rearrange("(c p) a -> p c a", p=128))
            qT_sb = setupp.tile([128, DEC // 128, bpc], bf16)
            nc.scalar.dma_start(qT_sb, qT.rearrange("(c p) b -> p c b", p=128))
            cb_sb = consts.tile([KCONV, bpc], bf16)
            nc.scalar.dma_start(cb_sb, cb[:])
            biasT_sb = consts.tile([128, AC], f32)
            nc.scalar.dma_start(biasT_sb, biasT[:])
            fcr_sb = consts.tile([128, AC], bf16)
            nc.scalar.dma_start(fcr_sb, fcr[:])

            ones_sb = consts.tile([1, 128], f32)
            nc.vector.memset(ones_sb, 1.0)
            ident_sb = consts.tile([128, 128], f32)
            from concourse.masks import make_identity
            make_identity(nc, ident_sb)

            # ---- W3T = cw^T @ WU^T : [3, ATTN] ----
            w3_ps = psmisc.tile([3, ATTN], f32, tag="mps")
            nc.tensor.matmul(w3_ps, lhsT=cw_sb, rhs=wut_sb, start=True, stop=True)
            w3_sb = consts.tile([3, ATTN], bf16)
            nc.scalar.copy(w3_sb, w3_ps)

            # ---- cT[a, chunk, b] = (query @ WQ^T + WU @ conv_b + bias)^T ----
            cT_sb = consts.tile([128, AC, bpc], f32)
            for a in range(AC):
                qt_ps = psmisc.tile([128, bpc], f32, tag="mps")
                for c in range(DEC // 128):
                    nc.tensor.matmul(
                        qt_ps,
                        lhsT=wqt_sb[:, c, a * 128:(a + 1) * 128],
                        rhs=qT_sb[:, c, :],
                        start=(c == 0),
                        stop=False,
                    )
                nc.tensor.matmul(
                    qt_ps,
                    lhsT=wut_sb[:, a * 128:(a + 1) * 128],
                    rhs=cb_sb,
                    start=False,
                    stop=True,
                )
                nc.scalar.activation(
                    cT_sb[:, a, :], qt_ps, AF.Identity,
                    bias=biasT_sb[:, a:a + 1], scale=1.0,
                )

            # ---- main loops ----
            HT = min(1024, t_len)       # columns per vT DMA tile
            spt = HT // TT              # subtiles per DMA tile
            n_ht = t_len // HT
            for b in range(bpc):
                la3_sb = la3p.tile([3, t_len], bf16)
                nc.sync.dma_start(la3_sb, la3[b])

                p_row = batchp.tile([1, t_len], f32, tag="p_row")
                p_dram = dramp.tile([t_len], f32, tag="p_dram")
                p_bdram = dramp.tile([t_len], bf16, tag="p_bdram")
                psums = batchp.tile([1, n_tiles], f32, tag="psums")
                parts = batchp.tile([128, EC, n_tiles], f32, tag="parts")

                ht_tiles = {}
                vT_r = vT[b].rearrange("(c p) t -> p c t", p=128)
                for h in range(n_ht):
                    vt_tile = vtp.tile([128, EC, HT], bf16, tag="vt",
                                       name=f"vt_{b}_{h}")
                    if b == 0 and h == 0:
                        half = HT // 2
                        nc.sync.dma_start(vt_tile[:, :, :half],
                                          vT_r[:, :, :half])
                        nc.sync.dma_start(vt_tile[:, :, half:HT],
                                          vT_r[:, :, half:HT])
                    else:
                        nc.sync.dma_start(
                            vt_tile, vT_r[:, :, h * HT:(h + 1) * HT]
                        )
                    ht_tiles[h] = vt_tile

                def vt_sl(ti):
                    """(tile, free-slice) of subtile ti's columns."""
                    h, off = divmod(ti * TT, HT)
                    return ht_tiles[h], slice(off, off + TT)

                # group plan: JJ subtiles share each stationary load; taper
                # the final groups of the last batch to shorten the drain.
                plan = []
                rem = n_tiles
                while rem > 0:
                    jj = min(4, rem)
                    plan.append(jj)
                    rem -= jj
                if b == bpc - 1 and plan and plan[-1] == 4:
                    plan = plan[:-1] + [2, 1, 1]

                s0 = 0
                for gi, jj in enumerate(plan):
                    subt = [s0 + j for j in range(jj)]
                    s0 += jj

                    tanhs = {}
                    for a in range(AC):
                        zs = []
                        for j, ti in enumerate(subt):
                            z_ps = psz.tile([128, TT], f32, tag="z",
                                            name=f"z_{b}_{gi}_{a}_{j}")
                            zs.append(z_ps)
                            nc.tensor.matmul(
                                z_ps,
                                lhsT=w3_sb[:, a * 128:(a + 1) * 128],
                                rhs=la3_sb[:, ti * TT:(ti + 1) * TT],
                                start=True,
                                stop=False,
                            )
                        for c in range(EC):
                            for j, ti in enumerate(subt):
                                vt_t, fsl = vt_sl(ti)
                                nc.tensor.matmul(
                                    zs[j],
                                    lhsT=wvt_sb[:, c, a * 128:(a + 1) * 128],
                                    rhs=vt_t[:, c, fsl],
                                    start=False,
                                    stop=(c == EC - 1),
                                )
                        for j, ti in enumerate(subt):
                            tanh_sb = tanhp.tile([128, TT], bf16, tag="tanh",
                                                 name=f"tanh_{b}_{gi}_{a}_{j}")
                            nc.scalar.activation(
                                tanh_sb, zs[j], AF.Tanh,
                                bias=cT_sb[:, a, b:b + 1], scale=1.0,
                            )
                            tanhs[(a, j)] = tanh_sb

                    for j, ti in enumerate(subt):
                        tsl = slice(ti * TT, (ti + 1) * TT)
                        s_ps = pss.tile([1, TT], f32, tag="s",
                                        name=f"s_{b}_{gi}_{j}")
                        for a in range(AC):
                            nc.tensor.matmul(
                                s_ps,
                                lhsT=fcr_sb[:, a:a + 1],
                                rhs=tanhs[(a, j)],
                                start=(a == 0),
                                stop=(a == AC - 1),
                            )

                        nc.scalar.activation(
                            p_row[0:1, tsl], s_ps, AF.Sigmoid,
                            accum_out=psums[0:1, ti:ti + 1],
                        )

                        pb_sb = pbp.tile([128, TT], bf16, tag="pb_sb",
                                         name=f"pb_{b}_{gi}_{j}")
                        if b == bpc - 1:
                            p_bf = pbp.tile([1, TT], bf16, tag="p_bf",
                                            name=f"pbf_{b}_{gi}_{j}")
                            nc.scalar.copy(p_bf, p_row[0:1, tsl])
                            nc.scalar.dma_start(
                                p_bdram[tsl][None, :], p_bf
                            )
                            nc.sync.dma_start(
                                pb_sb,
                                p_bdram[tsl][None, :].to_broadcast([128, TT]),
                            )
                        else:
                            nc.scalar.dma_start(
                                p_dram[tsl][None, :], p_row[0:1, tsl]
                            )
                            nc.gpsimd.dma_start(
                                pb_sb,
                                p_dram[tsl][None, :].to_broadcast([128, TT]),
                            )

                        scr = scrp.tile([128, TT], bf16, tag="scr",
                                        name=f"scr_{b}_{gi}_{j}")
                        vt_t, fsl = vt_sl(ti)
                        for c in range(EC):
                            nc.vector.scalar_tensor_tensor(
                                out=scr,
                                in0=vt_t[:, c, fsl],
                                scalar=1.0,
                                in1=pb_sb,
                                op0=ALU.mult,
                                op1=ALU.mult,
                                accum_out=parts[:, c, ti:ti + 1],
                            )

                # ---- batch epilogue ----
                sum1 = batchp.tile([1, 1], f32, tag="sum1")
                nc.vector.reduce_sum(sum1, psums, axis=mybir.AxisListType.X)
                inv1 = batchp.tile([1, 1], f32, tag="inv1")
                nc.vector.reciprocal(inv1, sum1)

                invp_sb = batchp.tile([128, 1], f32, tag="invp_sb")
                if b == bpc - 1:
                    invp_ps = psmisc.tile([128, 1], f32, tag="mps",
                                          name=f"invp_{b}")
                    nc.tensor.matmul(invp_ps, lhsT=ones_sb, rhs=inv1,
                                     start=True, stop=True)
                    nc.scalar.copy(invp_sb, invp_ps)
                else:
                    i_dram = dramp.tile([1], f32, tag="i_dram")
                    nc.gpsimd.dma_start(i_dram[None, :], inv1)
                    nc.gpsimd.dma_start(
                        invp_sb, i_dram[None, :].to_broadcast([128, 1])
                    )

                ctx_acc = batchp.tile([128, EC], f32, tag="ctx_acc")
                for c in range(EC):
                    nc.vector.reduce_sum(
                        ctx_acc[:, c:c + 1], parts[:, c, :],
                        axis=mybir.AxisListType.X,
                    )
                ctx_sb = batchp.tile([128, EC], f32, tag="ctx_sb")
                nc.vector.tensor_scalar_mul(ctx_sb, ctx_acc, invp_sb)
                if b == bpc - 1:
                    ctxT_ps = psmisc.tile([EC, 128], f32, tag="mps",
                                          name=f"ctxT_{b}")
                    nc.tensor.transpose(ctxT_ps, ctx_sb, ident_sb)
                    ctxT_sb = batchp.tile([EC, 128], f32, tag="ctxT_sb")
                    nc.scalar.copy(ctxT_sb, ctxT_ps)
                    nc.scalar.dma_start(
                        ctx_out[b].rearrange("(c p) -> c p", p=128), ctxT_sb
                    )
                else:
                    nc.gpsimd.dma_start(
                        ctx_out[b].rearrange("(c p) -> p c", p=128), ctx_sb
                    )

                nc.vector.tensor_scalar_mul(p_row, p_row, inv1)
                if b == bpc - 1:
                    nc.scalar.dma_start(align_out[b:b + 1, :], p_row)
                else:
                    nc.gpsimd.dma_start(align_out[b:b + 1, :], p_row)

    nc.compile()
    return nc


def prep_inputs(query, value, last_align, conv_w, conv_b, WQ, WV, WU, bias, fc_w,
                bpc=BPC, n_cores=N_CORES):
    """Host-side sharding + layout prep. Returns list of per-core input dicts."""
    t_len = value.shape[1]

    value = np.asarray(value, np.float32)
    la = np.asarray(last_align, np.float32)

    # value^T per batch, bf16
    vT_all = np.ascontiguousarray(value.transpose(0, 2, 1)).astype(BF16)

    nb = vT_all.shape[0]
    la3_all = np.zeros((nb, 3, t_len), np.float32)
    la3_all[:, 0, 1:] = la[:, :-1]
    la3_all[:, 1, :] = la
    la3_all[:, 2, :-1] = la[:, 1:]
    la3_all = la3_all.astype(BF16)

    wvt_h = np.ascontiguousarray(np.asarray(WV, np.float32).T).astype(BF16)
    wqt_h = np.ascontiguousarray(np.asarray(WQ, np.float32).T).astype(BF16)
    wut_h = np.ascontiguousarray(np.asarray(WU, np.float32).T).astype(BF16)
    cw_h = np.ascontiguousarray(np.asarray(conv_w, np.float32)[:, 0, :]).astype(BF16)
    biasT_h = np.ascontiguousarray(
        np.asarray(bias, np.float32).reshape(AC, 128).T
    ).astype(np.float32)
    fcr_h = np.ascontiguousarray(
        np.asarray(fc_w, np.float32)[0].reshape(AC, 128).T
    ).astype(BF16)

    qT_all = np.ascontiguousarray(
        np.asarray(query, np.float32)[:, 0, :].T
    ).astype(BF16)  # [DEC, B]
    cb_h = np.ascontiguousarray(
        np.repeat(np.asarray(conv_b, np.float32)[:, None], bpc, axis=1)
    ).astype(BF16)

    in_maps = []
    for c in range(n_cores):
        bs = slice(c * bpc, (c + 1) * bpc)
        in_maps.append({
            "vT": np.ascontiguousarray(vT_all[bs]),
            "la3": np.ascontiguousarray(la3_all[bs]),
            "wvt": wvt_h,
            "wqt": wqt_h,
            "qT": np.ascontiguousarray(qT_all[:, bs]),
            "wut": wut_h,
            "cw": cw_h,
            "cb": cb_h,
            "biasT": biasT_h,
            "fcr": fcr_h,
        })
    return in_maps


@functools.lru_cache(maxsize=1)
def _get_nc():
    return build_kernel()


def run(inputs, trace=False, **kwargs):
    from concourse.bass_utils import run_bass_kernel_spmd

    nc = _get_nc()
    in_maps = prep_inputs(**inputs)
    res = run_bass_kernel_spmd(
        nc, in_maps, core_ids=list(range(N_CORES)), trace=trace, **kwargs
    )
    ctx = np.concatenate([np.asarray(r["ctx_out"]) for r in res.results], axis=0)
    align = np.concatenate([np.asarray(r["align_out"]) for r in res.results], axis=0)
    return (ctx.astype(np.float32), align.astype(np.float32)), res


def kernel(**inputs):
    (ctx, align), _ = run(inputs, trace=False)
    return ctx, align


# revision 25
# speedup vs baseline: 1.4220x; 1.0152x over previous
"""LocationAwareAttention Trainium2 kernel.

Full-input contract: kernel(**inputs) takes the complete unsharded inputs
(as produced by the problem's setup_inputs) and returns (context, align) as
full-shape fp32 arrays.  Internally the batch dimension (B=32) is sharded
across 8 NeuronCores (4 batches per core); all weights are replicated.

Math (per batch b):
    conv_feat = conv1d(last_align, conv_w, pad=1) + conv_b          [T, K]
    z[t, a]   = (value[t] @ WV^T)[a] + (conv_feat[t] @ WU^T)[a]
                + (query @ WQ^T)[a] + bias[a]
    score[t]  = fc_w . tanh(z[t])
    p         = sigmoid(score);  align = p / sum(p)
    context   = align @ value                                        [ENC]

Device-side layout (per core, per batch):
  - value is staged host-side as value^T (bf16, [ENC, T]) so the big
    projection runs with the contraction dim (ENC) on partitions.
  - conv+WU collapse into a rank-3 term: u[t,:] = sum_j W3[:,j]*la[t+j-1],
    where W3 = WU @ conv_w — computed on device from WU^T and conv_w.
  - z tiles are [a_chunk(128), t(512)] psum banks: 1 K=3 matmul (u term)
    + 8 K=128 matmuls (value^T @ WV^T chunks); per-batch constant
    q + bias + WU@conv_b enters via the tanh activation's per-partition bias.
  - score: 4 fc matmuls (M=1) accumulate into psum [1, 512]; sigmoid (ACT)
    with accum_out produces the per-tile sum of p for free.
  - context: p is bounced through DRAM to broadcast it across all 128
    partitions (with an fp32->bf16 cast in the DMA), then one fused DVE
    scalar_tensor_tensor per e-chunk does the multiply and row-reduce into
    per-subtile partials; a per-batch reduce + 1/sum scaling finishes it.
"""

import os
import sys
import functools

_TRN_REPO = "/opt/trn_rl_repo"
if _TRN_REPO not in sys.path and os.path.isdir(_TRN_REPO):
    sys.path.insert(0, _TRN_REPO)

import numpy as np
import ml_dtypes

BF16 = ml_dtypes.bfloat16

B, T_FULL, DEC, ENC, ATTN, KCONV = 32, 4096, 1024, 1024, 512, 10
N_CORES = 8
BPC = B // N_CORES          # batches per core
TT = 512                    # t-tile (columns per psum bank)
EC = ENC // 128             # e chunks (8)
AC = ATTN // 128            # a chunks (4)


def build_kernel(bpc=BPC, t_len=T_FULL):
    """Build the Bass module for one core handling `bpc` batches of length t_len."""
    import concourse.bass as bass  # noqa: F401
    import concourse.tile as tile
    from concourse import bacc, mybir

    f32 = mybir.dt.float32
    bf16 = mybir.dt.bfloat16
    AF = mybir.ActivationFunctionType
    ALU = mybir.AluOpType

    n_tiles = t_len // TT

    nc = bacc.Bacc(trn_type="TRN2")

    # ---- DRAM I/O ----
    vT = nc.dram_tensor("vT", [bpc, ENC, t_len], bf16, kind="ExternalInput")
    la3 = nc.dram_tensor("la3", [bpc, 3, t_len], bf16, kind="ExternalInput")
    wvt = nc.dram_tensor("wvt", [ENC, ATTN], bf16, kind="ExternalInput")
    wqt = nc.dram_tensor("wqt", [DEC, ATTN], bf16, kind="ExternalInput")
    qT = nc.dram_tensor("qT", [DEC, bpc], bf16, kind="ExternalInput")
    wut = nc.dram_tensor("wut", [KCONV, ATTN], bf16, kind="ExternalInput")
    cw = nc.dram_tensor("cw", [KCONV, 3], bf16, kind="ExternalInput")
    cb = nc.dram_tensor("cb", [KCONV, bpc], bf16, kind="ExternalInput")
    biasT = nc.dram_tensor("biasT", [128, AC], f32, kind="ExternalInput")
    fcr = nc.dram_tensor("fcr", [128, AC], bf16, kind="ExternalInput")

    ctx_out = nc.dram_tensor("ctx_out", [bpc, ENC], f32, kind="ExternalOutput")
    align_out = nc.dram_tensor("align_out", [bpc, t_len], f32, kind="ExternalOutput")

    with tile.TileContext(nc) as tc:
        with (
            tc.tile_pool(name="consts", bufs=1) as consts,
            tc.tile_pool(name="setupp", bufs=1) as setupp,
            tc.tile_pool(name="vtp", bufs=6) as vtp,
            tc.tile_pool(name="la3p", bufs=2) as la3p,
            tc.tile_pool(name="tanhp", bufs=20) as tanhp,
            tc.tile_pool(name="pbp", bufs=4) as pbp,
            tc.tile_pool(name="scrp", bufs=2) as scrp,
            tc.tile_pool(name="batchp", bufs=2) as batchp,
            tc.tile_pool(name="dramp", bufs=2, space="DRAM") as dramp,
            tc.tile_pool(name="psz", bufs=6, space="PSUM") as psz,
            tc.tile_pool(name="pss", bufs=1, space="PSUM") as pss,
            tc.tile_pool(name="psmisc", bufs=1, space="PSUM") as psmisc,
        ):
            # ---- constant loads ----
            wvt_sb = consts.tile([128, EC, ATTN], bf16)
            wvt_r = wvt.rearrange("(c p) a -> p c a", p=128)
            nc.sync.dma_start(wvt_sb[:, :2, :], wvt_r[:, :2, :])
            nc.sync.dma_start(wvt_sb[:, 2:EC, :], wvt_r[:, 2:EC, :])
            wut_sb = consts.tile([KCONV, ATTN], bf16)
            nc.scalar.dma_start(wut_sb, wut[:])
            cw_sb = consts.tile([KCONV, 3], bf16)
            nc.scalar.dma_start(cw_sb, cw[:])
            wqt_sb = setupp.tile([128, DEC // 128, ATTN], bf16)
            nc.scalar.dma_start(wqt_sb, wqt.rearrange("(c p) a -> p c a", p=128))
            qT_sb = setupp.tile([128, DEC // 128, bpc], bf16)
            nc.scalar.dma_start(qT_sb, qT.rearrange("(c p) b -> p c b", p=128))
            cb_sb = consts.tile([KCONV, bpc], bf16)
            nc.scalar.dma_start(cb_sb, cb[:])
            biasT_sb = consts.tile([128, AC], f32)
            nc.scalar.dma_start(biasT_sb, biasT[:])
            fcr_sb = consts.tile([128, AC], bf16)
            nc.scalar.dma_start(fcr_sb, fcr[:])

            ones_sb = consts.tile([1, 128], f32)
            nc.vector.memset(ones_sb, 1.0)
            ident_sb = consts.tile([128, 128], f32)
            from concourse.masks import make_identity
            make_identity(nc, ident_sb)

            # ---- W3T = cw^T @ WU^T : [3, ATTN] ----
            w3_ps = psmisc.tile([3, ATTN], f32, tag="mps")
            nc.tensor.matmul(w3_ps, lhsT=cw_sb, rhs=wut_sb, start=True, stop=True)
            w3_sb = consts.tile([3, ATTN], bf16)
            nc.scalar.copy(w3_sb, w3_ps)

            # ---- cT[a, chunk, b] = (query @ WQ^T + WU @ conv_b + bias)^T ----
            cT_sb = consts.tile([128, AC, bpc], f32)
            for a in range(AC):
                qt_ps = psmisc.tile([128, bpc], f32, tag="mps")
                for c in range(DEC // 128):
                    nc.tensor.matmul(
                        qt_ps,
                        lhsT=wqt_sb[:, c, a * 128:(a + 1) * 128],
                        rhs=qT_sb[:, c, :],
                        start=(c == 0),
                        stop=False,
                    )
                nc.tensor.matmul(
                    qt_ps,
                    lhsT=wut_sb[:, a * 128:(a + 1) * 128],
                    rhs=cb_sb,
                    start=False,
                    stop=True,
                )
                nc.scalar.activation(
                    cT_sb[:, a, :], qt_ps, AF.Identity,
                    bias=biasT_sb[:, a:a + 1], scale=1.0,
                )

            # ---- main loops ----
            HT = min(1024, t_len)       # columns per vT DMA tile
            spt = HT // TT              # subtiles per DMA tile
            n_ht = t_len // HT
            for b in range(bpc):
                la3_sb = la3p.tile([3, t_len], bf16)
                nc.sync.dma_start(la3_sb, la3[b])

                p_row = batchp.tile([1, t_len], f32, tag="p_row")
                p_dram = dramp.tile([t_len], f32, tag="p_dram")
                p_bdram = dramp.tile([t_len], bf16, tag="p_bdram")
                psums = batchp.tile([1, n_tiles], f32, tag="psums")
                parts = batchp.tile([128, EC, n_tiles], f32, tag="parts")

                ht_tiles = {}
                vT_r = vT[b].rearrange("(c p) t -> p c t", p=128)
                for h in range(n_ht):
                    vt_tile = vtp.tile([128, EC, HT], bf16, tag="vt",
                                       name=f"vt_{b}_{h}")
                    if b == 0 and h == 0:
                        half = HT // 2
                        nc.sync.dma_start(vt_tile[:, :, :half],
                                          vT_r[:, :, :half])
                        nc.sync.dma_start(vt_tile[:, :, half:HT],
                                          vT_r[:, :, half:HT])
                    else:
                        nc.sync.dma_start(
                            vt_tile, vT_r[:, :, h * HT:(h + 1) * HT]
                        )
                    ht_tiles[h] = vt_tile

                def vt_sl(ti):
                    """(tile, free-slice) of subtile ti's columns."""
                    h, off = divmod(ti * TT, HT)
                    return ht_tiles[h], slice(off, off + TT)

                # group plan: JJ subtiles share each stationary load; taper
                # the final groups of the last batch to shorten the drain.
                plan = []
                rem = n_tiles
                while rem > 0:
                    jj = min(4, rem)
                    plan.append(jj)
                    rem -= jj
                if b == bpc - 1 and plan and plan[-1] == 4:
                    plan = plan[:-1] + [2, 1, 1]

                s0 = 0
                for gi, jj in enumerate(plan):
                    subt = [s0 + j for j in range(jj)]
                    s0 += jj

                    tanhs = {}
                    for a in range(AC):
                        zs = []
                        for j, ti in enumerate(subt):
                            z_ps = psz.tile([128, TT], f32, tag="z",
                                            name=f"z_{b}_{gi}_{a}_{j}")
                            zs.append(z_ps)
                            nc.tensor.matmul(
                                z_ps,
                                lhsT=w3_sb[:, a * 128:(a + 1) * 128],
                                rhs=la3_sb[:, ti * TT:(ti + 1) * TT],
                                start=True,
                                stop=False,
                            )
                        for c in range(EC):
                            for j, ti in enumerate(subt):
                                vt_t, fsl = vt_sl(ti)
                                nc.tensor.matmul(
                                    zs[j],
                                    lhsT=wvt_sb[:, c, a * 128:(a + 1) * 128],
                                    rhs=vt_t[:, c, fsl],
                                    start=False,
                                    stop=(c == EC - 1),
                                )
                        for j, ti in enumerate(subt):
                            tanh_sb = tanhp.tile([128, TT], bf16, tag="tanh",
                                                 name=f"tanh_{b}_{gi}_{a}_{j}")
                            nc.scalar.activation(
                                tanh_sb, zs[j], AF.Tanh,
                                bias=cT_sb[:, a, b:b + 1], scale=1.0,
                            )
                            tanhs[(a, j)] = tanh_sb

                    for j, ti in enumerate(subt):
                        tsl = slice(ti * TT, (ti + 1) * TT)
                        s_ps = pss.tile([1, TT], f32, tag="s",
                                        name=f"s_{b}_{gi}_{j}")
                        for a in range(AC):
                            nc.tensor.matmul(
                                s_ps,
                                lhsT=fcr_sb[:, a:a + 1],
                                rhs=tanhs[(a, j)],
                                start=(a == 0),
                                stop=(a == AC - 1),
                            )

                        nc.scalar.activation(
                            p_row[0:1, tsl], s_ps, AF.Sigmoid,
                            accum_out=psums[0:1, ti:ti + 1],
                        )

                        pb_sb = pbp.tile([128, TT], bf16, tag="pb_sb",
                                         name=f"pb_{b}_{gi}_{j}")
                        if b == bpc - 1:
                            p_bf = pbp.tile([1, TT], bf16, tag="p_bf",
                                            name=f"pbf_{b}_{gi}_{j}")
                            nc.scalar.copy(p_bf, p_row[0:1, tsl])
                            nc.scalar.dma_start(
                                p_bdram[tsl][None, :], p_bf
                            )
                            nc.sync.dma_start(
                                pb_sb,
                                p_bdram[tsl][None, :].to_broadcast([128, TT]),
                            )
                        else:
                            nc.scalar.dma_start(
                                p_dram[tsl][None, :], p_row[0:1, tsl]
                            )
                            nc.gpsimd.dma_start(
                                pb_sb,
                                p_dram[tsl][None, :].to_broadcast([128, TT]),
                            )

                        scr = scrp.tile([128, TT], bf16, tag="scr",
                                        name=f"scr_{b}_{gi}_{j}")
                        vt_t, fsl = vt_sl(ti)
                        for c in range(EC):
                            nc.vector.scalar_tensor_tensor(
                                out=scr,
                                in0=vt_t[:, c, fsl],
                                scalar=1.0,
                                in1=pb_sb,
                                op0=ALU.mult,
                                op1=ALU.mult,
                                accum_out=parts[:, c, ti:ti + 1],
                            )

                # ---- batch epilogue ----
                sum1 = batchp.tile([1, 1], f32, tag="sum1")
                nc.vector.reduce_sum(sum1, psums, axis=mybir.AxisListType.X)
                inv1 = batchp.tile([1, 1], f32, tag="inv1")
                nc.vector.reciprocal(inv1, sum1)

                invp_sb = batchp.tile([128, 1], f32, tag="invp_sb")
                if b == bpc - 1:
                    invp_ps = psmisc.tile([128, 1], f32, tag="mps",
                                          name=f"invp_{b}")
                    nc.tensor.matmul(invp_ps, lhsT=ones_sb, rhs=inv1,
                                     start=True, stop=True)
                    nc.scalar.copy(invp_sb, invp_ps)
                else:
                    i_dram = dramp.tile([1], f32, tag="i_dram")
                    nc.gpsimd.dma_start(i_dram[None, :], inv1)
                    nc.gpsimd.dma_start(
                        invp_sb, i_dram[None, :].to_broadcast([128, 1])
                    )

                ctx_acc = batchp.tile([128, EC], f32, tag="ctx_acc")
                for c in range(EC):
                    nc.vector.reduce_sum(
                        ctx_acc[:, c:c + 1], parts[:, c, :],
                        axis=mybir.AxisListType.X,
                    )
                ctx_sb = batchp.tile([128, EC], f32, tag="ctx_sb")
                nc.vector.tensor_scalar_mul(ctx_sb, ctx_acc, invp_sb)
                if b == bpc - 1:
                    ctxT_ps = psmisc.tile([EC, 128], f32, tag="mps",
                                          name=f"ctxT_{b}")
                    nc.tensor.transpose(ctxT_ps, ctx_sb, ident_sb)
                    ctxT_sb = batchp.tile([EC, 128], f32, tag="ctxT_sb")
                    nc.scalar.copy(ctxT_sb, ctxT_ps)
                    nc.scalar.dma_start(
                        ctx_out[b].rearrange("(c p) -> c p", p=128), ctxT_sb
                    )
                else:
                    nc.gpsimd.dma_start(
                        ctx_out[b].rearrange("(c p) -> p c", p=128), ctx_sb
                    )

                half_t = t_len // 2
                nc.vector.tensor_scalar_mul(
                    p_row[0:1, :half_t], p_row[0:1, :half_t], inv1
                )
                nc.scalar.activation(
                    p_row[0:1, half_t:], p_row[0:1, half_t:], AF.Copy,
                    scale=inv1[0:1, 0:1]
                )
                if b == bpc - 1:
                    nc.scalar.dma_start(align_out[b:b + 1, :], p_row)
                else:
                    nc.gpsimd.dma_start(align_out[b:b + 1, :], p_row)

    nc.compile()
    return nc


def prep_inputs(query, value, last_align, conv_w, conv_b, WQ, WV, WU, bias, fc_w,
                bpc=BPC, n_cores=N_CORES):
    """Host-side sharding + layout prep. Returns list of per-core input dicts."""
    t_len = value.shape[1]

    value = np.asarray(value, np.float32)
    la = np.asarray(last_align, np.float32)

    # value^T per batch, bf16
    vT_all = np.ascontiguousarray(value.transpose(0, 2, 1)).astype(BF16)

    nb = vT_all.shape[0]
    la3_all = np.zeros((nb, 3, t_len), np.float32)
    la3_all[:, 0, 1:] = la[:, :-1]
    la3_all[:, 1, :] = la
    la3_all[:, 2, :-1] = la[:, 1:]
    la3_all = la3_all.astype(BF16)

    wvt_h = np.ascontiguousarray(np.asarray(WV, np.float32).T).astype(BF16)
    wqt_h = np.ascontiguousarray(np.asarray(WQ, np.float32).T).astype(BF16)
    wut_h = np.ascontiguousarray(np.asarray(WU, np.float32).T).astype(BF16)
    cw_h = np.ascontiguousarray(np.asarray(conv_w, np.float32)[:, 0, :]).astype(BF16)
    biasT_h = np.ascontiguousarray(
        np.asarray(bias, np.float32).reshape(AC, 128).T
    ).astype(np.float32)
    fcr_h = np.ascontiguousarray(
        np.asarray(fc_w, np.float32)[0].reshape(AC, 128).T
    ).astype(BF16)

    qT_all = np.ascontiguousarray(
        np.asarray(query, np.float32)[:, 0, :].T
    ).astype(BF16)  # [DEC, B]
    cb_h = np.ascontiguousarray(
        np.repeat(np.asarray(conv_b, np.float32)[:, None], bpc, axis=1)
    ).astype(BF16)

    in_maps = []
    for c in range(n_cores):
        bs = slice(c * bpc, (c + 1) * bpc)
        in_maps.append({
            "vT": np.ascontiguousarray(vT_all[bs]),
            "la3": np.ascontiguousarray(la3_all[bs]),
            "wvt": wvt_h,
            "wqt": wqt_h,
            "qT": np.ascontiguousarray(qT_all[:, bs]),
            "wut": wut_h,
            "cw": cw_h,
            "cb": cb_h,
            "biasT": biasT_h,
            "fcr": fcr_h,
        })
    return in_maps


@functools.lru_cache(maxsize=1)
def _get_nc():
    return build_kernel()


def run(inputs, trace=False, **kwargs):
    from concourse.bass_utils import run_bass_kernel_spmd

    nc = _get_nc()
    in_maps = prep_inputs(**inputs)
    res = run_bass_kernel_spmd(
        nc, in_maps, core_ids=list(range(N_CORES)), trace=trace, **kwargs
    )
    ctx = np.concatenate([np.asarray(r["ctx_out"]) for r in res.results], axis=0)
    align = np.concatenate([np.asarray(r["align_out"]) for r in res.results], axis=0)
    return (ctx.astype(np.float32), align.astype(np.float32)), res


def kernel(**inputs):
    (ctx, align), _ = run(inputs, trace=False)
    return ctx, align
